# revision 1
# baseline (speedup 1.0000x reference)
"""Trainium2 Bass kernel for nn_NetworkAwareClassicalExpert (B=256,T=363,C=180).

Data-parallel over 8 NeuronCores: 32 samples/core. Per-core program processes
samples in a feature-major (channels-on-partitions) layout, transposing to
token-major for layernorm/softmax row ops.

Key tricks:
  - depthwise dilated conv folded into the pointwise conv: 7 shifted matmuls
    (rhs AP offset into a zero-padded activation tile) accumulating in PSUM.
  - attention: scores computed transposed [s,t]; exp via ACT; AV as small
    matmuls producing token-major output with an appended ones-column in the
    rhs giving the softmax normalizer Z for free; normalization is then a
    per-partition scale in token-major layout.
  - FC fingerprint: block means of the 180x180 correlation collapse to a
    12x12 Gram of per-network invstd-weighted channel sums + rank-1 mean
    correction.
"""

import sys
import os

sys.path.insert(0, "/opt/trn_rl_repo")

import numpy as np

import concourse.bass as bass
import concourse.mybir as mybir
import concourse.tile as tile
import bass_rust
from concourse.vector_clock import ScopedClock
from concourse.masks import make_identity

F32 = mybir.dt.float32
F32R = mybir.dt.float32r
AF = mybir.ActivationFunctionType
ALU = mybir.AluOpType

B, T, C = 256, 363, 180
CD = 96
DM = 128
DILS = (1, 4, 16)
NCORES = 8
S = int(os.environ.get("KB_NSAMP", str(B // NCORES)))
TCH = 121                # t-chunk (3 chunks of 121)
PAD = 48
EPS = 1e-5
ISQ = float(1.0 / np.sqrt(32.0))

DEBUG = bool(int(os.environ.get("KBDBG", "0")))
DBG_SAMPLES = int(os.environ.get("KBDBG_S", "2"))


def _patch_tile_drain():
    """This walrus rejects >1 sem wait on the final Tile drain: split them."""

    def _drain_and_barrier(self, tick_clock, wait_clock):
        drain_inst = self.nc.sync.drain()
        wait_clock.add_sem_waits(
            drain_inst.ins, ScopedClock({None: tick_clock.global_clock})
        )
        si = drain_inst.ins.sync_info
        if si is not None and si.on_wait is not None and len(si.on_wait) > 1:
            waits = list(si.on_wait)
            ups = list(si.on_update) if si.on_update else []
            drain_inst.ins.sync_info = bass_rust.SyncInfo(
                on_wait=waits[:1], on_update=ups
            )
            for w in waits[1:]:
                nop = self.nc.sync.nop()
                nop.ins.sync_info = bass_rust.SyncInfo(on_wait=[w], on_update=[])
        self.nc.all_engine_barrier()
        popped = self.nc._tile_sem_poison_stack.pop()
        assert popped is self._sem_poison
        if not int(os.environ.get("KB_NOSEMCLEAR", "0")):
            self.nc.clear_and_free_semaphores(list(self.sems.allocated().values()))
        self.nc.all_engine_barrier()

    tile.TileContext._drain_and_barrier = _drain_and_barrier


_patch_tile_drain()


def r32(ap):
    return ap.bitcast(F32R)


_NOPN = [0]


def _split_sync_waits(nc, max_waits=1):
    """walrus rejects instructions with >1 sem wait; hoist excess onto
    same-engine NOPs inserted immediately before (created via the real
    bass emitters so they carry all rust-side init)."""
    for f in nc.m.functions:
        for bb in f.blocks:
            insts = list(bb.instructions)
            out = []
            changed = False
            for inst in insts:
                si = getattr(inst, "sync_info", None)
                if si is not None and si.on_wait and len(si.on_wait) > max_waits:
                    waits = list(si.on_wait)
                    ups = list(si.on_update) if si.on_update else []
                    extra = waits[max_waits:]
                    for i in range(0, len(extra), max_waits):
                        nop = nc.engines[inst.engine].nop(nofuse=True)
                        # pop it from wherever add_instruction appended it
                        cur = nn_cur_bb(nc)
                        popped = cur.instructions
                        lst = list(popped)
                        assert lst and lst[-1].name == nop.ins.name
                        cur.instructions = lst[:-1]
                        nop.ins.sync_info = bass_rust.SyncInfo(
                            on_wait=extra[i:i + max_waits], on_update=[])
                        out.append(nop.ins)
                    inst.sync_info = bass_rust.SyncInfo(
                        on_wait=waits[:max_waits], on_update=ups)
                    changed = True
                out.append(inst)
            if changed:
                bb.instructions = out


def nn_cur_bb(nc):
    bbw = nc.cur_bb
    return bbw.bb if hasattr(bbw, "bb") else bbw


INPUT_SPECS = [
    ("x", (S, T, C)),
    ("w_proj", (12, 15, 8)), ("b_proj", (12, 8)),
    ("dw_w", (3, 96, 7)), ("dw_b", (3, 96)),
    ("pw_w", (3, 96, 96)), ("pw_b", (3, 96)),
    ("gn_g", (3, 96)), ("gn_b", (3, 96)),
    ("merge_w", (288, 128)), ("merge_b", (128,)),
    ("merge_ln_g", (128,)), ("merge_ln_b", (128,)),
    ("qkv_w", (384, 128)), ("qkv_b", (384,)),
    ("out_w", (128, 128)), ("out_b", (128,)),
    ("ln1_g", (128,)), ("ln1_b", (128,)),
    ("ff1_w", (256, 128)), ("ff1_b", (256,)),
    ("ff2_w", (128, 256)), ("ff2_b", (128,)),
    ("ln2_g", (128,)), ("ln2_b", (128,)),
    ("pool_w", (128, 1)), ("pool_b", (1,)),
    ("fc_w", (78, 64)), ("fc_b", (64,)),
    ("fus1_w", (192, 128)), ("fus1_b", (128,)),
    ("fus_ln_g", (128,)), ("fus_ln_b", (128,)),
    ("fus2_w", (128, 64)), ("fus2_b", (64,)),
]


def build_program():
    nc = bass.Bass("TRN2", target_bir_lowering=False, debug=False,
                   num_devices=NCORES)
    D = {}
    for name, shape in INPUT_SPECS:
        D[name] = nc.dram_tensor(name, list(shape), F32, kind="ExternalInput").ap()
    out_dram = nc.dram_tensor("out", [S, 64], F32, kind="ExternalOutput").ap()
    dbg_shapes = {}

    with tile.TileContext(nc) as tc:
        _build(nc, tc, D, out_dram, dbg_shapes)
    if not int(os.environ.get("KB_NOSPLIT", "0")):
        _split_sync_waits(nc)
    return nc, dbg_shapes


def _build(nc, tc, D, out_dram, dbg_shapes):
    pools = []

    def mkpool(name, bufs, space="SBUF"):
        p = tc.tile_pool(name=name, bufs=bufs, space=space)
        pools.append(p)
        return p.__enter__()

    W = mkpool("weights", 1)        # persistent tiles, one tag each
    sb = mkpool("sb", 2)            # per-sample activations
    tiny = mkpool("tiny", 3)        # small per-sample stats
    ppA = mkpool("ppA", 4, "PSUM")  # all 1-bank psum tiles, shared tag "a"
    ppB = mkpool("ppB", 1, "PSUM")  # 4-bank psum (conv / scores), tag "big"

    dma = nc.sync

    _pa_n = [0]

    def pa(shape):
        _pa_n[0] += 1
        return ppA.tile(list(shape), F32, tag="a", name=f"pa{_pa_n[0]}")

    def dbg(name, ap, shape):
        if not DEBUG:
            return
        t = nc.dram_tensor(f"dbg_{name}", list(shape), F32,
                           kind="ExternalOutput").ap()
        dbg_shapes[name] = tuple(shape)
        dma.dma_start(out=t, in_=ap)

    # ================= weight preload =================
    ident = W.tile([128, 128], F32, tag="ident")
    make_identity(nc, ident)

    wproj = W.tile([90, 2, 96], F32R, tag="wproj")
    nc.vector.memset(wproj.bitcast(F32), 0.0)
    for n in range(12):
        g, j = divmod(n, 6)
        dma.dma_start(out=wproj[j * 15:(j + 1) * 15, g, n * 8:(n + 1) * 8],
                      in_=D["w_proj"][n].bitcast(F32R))
    bproj = W.tile([96, 1], F32, tag="bproj")
    dma.dma_start(out=bproj, in_=D["b_proj"].rearrange("a b -> (a b)").unsqueeze(1))

    pwT, dwk = [], []
    for k in range(3):
        t_ = W.tile([96, 96], F32, tag=f"pwT{k}")
        dma.dma_start(out=t_, in_=D["pw_w"][k].transpose([1, 0]))
        pwT.append(t_)
        t2 = W.tile([96, 7], F32, tag=f"dw{k}")
        dma.dma_start(out=t2, in_=D["dw_w"][k])
        dwk.append(t2)
    wconv = []
    for k in range(3):
        t_ = W.tile([96, 7, 96], F32R, tag=f"wconv{k}")
        for j in range(7):
            nc.vector.tensor_scalar_mul(t_[:, j, :], pwT[k], dwk[k][:, j:j + 1])
        wconv.append(t_)
    dwb = W.tile([96, 3], F32, tag="dwb")
    dma.dma_start(out=dwb, in_=D["dw_b"].transpose([1, 0]))
    pwb = W.tile([96, 3], F32, tag="pwb")
    dma.dma_start(out=pwb, in_=D["pw_b"].transpose([1, 0]))
    cb_ps = pa([96, 3])
    for k in range(3):
        nc.tensor.matmul(cb_ps[:, k:k + 1], pwT[k], dwb[:, k:k + 1],
                         start=True, stop=True, skip_group_check=True)
    cb = W.tile([96, 3], F32, tag="cb")
    nc.vector.tensor_add(cb, cb_ps, pwb)
    cbT_ps = pa([1, 3, 96])
    for k in range(3):
        nc.tensor.transpose(cbT_ps[:, k, :], cb[:, k:k + 1], ident[0:96, 0:96])
    cbT = W.tile([1, 3, 96], F32R, tag="cbT")
    nc.vector.tensor_copy(cbT, cbT_ps)
    ones_row = W.tile([1, 364], F32R, tag="ones_row")
    nc.vector.memset(ones_row.bitcast(F32), 1.0)

    gng = W.tile([96, 3], F32, tag="gng")
    dma.dma_start(out=gng, in_=D["gn_g"].transpose([1, 0]))
    gnb = W.tile([96, 3], F32, tag="gnb")
    dma.dma_start(out=gnb, in_=D["gn_b"].transpose([1, 0]))

    # wgrp[c, g] = 1/(12T) iff 0 <= c - 12g <= 11 ; wbc[g, c] = 1 iff same
    wgrp = W.tile([96, 8], F32, tag="wgrp")
    nc.vector.memset(wgrp, 1.0 / 12.0)
    nc.gpsimd.affine_select(out=wgrp, in_=wgrp, compare_op=ALU.is_ge,
                            fill=0.0, base=0, pattern=[[-12, 8]],
                            channel_multiplier=1)
    nc.gpsimd.affine_select(out=wgrp, in_=wgrp, compare_op=ALU.is_ge,
                            fill=0.0, base=11, pattern=[[12, 8]],
                            channel_multiplier=-1)
    wbc = W.tile([8, 96], F32, tag="wbc")
    nc.vector.memset(wbc, 1.0)
    nc.gpsimd.affine_select(out=wbc, in_=wbc, compare_op=ALU.is_ge,
                            fill=0.0, base=0, pattern=[[1, 96]],
                            channel_multiplier=-12)
    nc.gpsimd.affine_select(out=wbc, in_=wbc, compare_op=ALU.is_ge,
                            fill=0.0, base=11, pattern=[[-1, 96]],
                            channel_multiplier=12)

    mw = []
    for g in range(3):
        t_ = W.tile([96, 128], F32R, tag=f"mw{g}")
        dma.dma_start(out=t_, in_=D["merge_w"][g * 96:(g + 1) * 96, :].bitcast(F32R))
        mw.append(t_)
    mb = W.tile([128, 1], F32, tag="mb")
    dma.dma_start(out=mb, in_=D["merge_b"].unsqueeze(1))
    mlng = W.tile([128, 1], F32, tag="mlng")
    dma.dma_start(out=mlng, in_=D["merge_ln_g"].unsqueeze(1))
    mlnb = W.tile([128, 1], F32, tag="mlnb")
    dma.dma_start(out=mlnb, in_=D["merge_ln_b"].unsqueeze(1))

    qkvT = []
    for i in range(3):
        t_ = W.tile([128, 128], F32R, tag=f"qkvT{i}")
        dma.dma_start(out=t_,
                      in_=D["qkv_w"][i * 128:(i + 1) * 128, :].transpose([1, 0]).bitcast(F32R))
        qkvT.append(t_)
    qb3 = W.tile([128, 3], F32, tag="qb3")
    dma.dma_start(out=qb3, in_=D["qkv_b"].rearrange("(a b) -> b a", a=3))
    qb_s = W.tile([128, 1], F32, tag="qb_s")
    nc.vector.tensor_scalar_mul(qb_s, qb3[:, 0:1], ISQ)

    owT = W.tile([128, 128], F32R, tag="owT")
    dma.dma_start(out=owT, in_=D["out_w"].transpose([1, 0]).bitcast(F32R))
    ob = W.tile([128, 1], F32, tag="ob")
    dma.dma_start(out=ob, in_=D["out_b"].unsqueeze(1))

    ln1g = W.tile([128, 1], F32, tag="ln1g")
    dma.dma_start(out=ln1g, in_=D["ln1_g"].unsqueeze(1))
    ln1b = W.tile([128, 1], F32, tag="ln1b")
    dma.dma_start(out=ln1b, in_=D["ln1_b"].unsqueeze(1))
    ln2g = W.tile([128, 1], F32, tag="ln2g")
    dma.dma_start(out=ln2g, in_=D["ln2_g"].unsqueeze(1))
    ln2b = W.tile([128, 1], F32, tag="ln2b")
    dma.dma_start(out=ln2b, in_=D["ln2_b"].unsqueeze(1))

    f1T, f2T = [], []
    for i in range(2):
        t_ = W.tile([128, 128], F32R, tag=f"f1T{i}")
        dma.dma_start(out=t_,
                      in_=D["ff1_w"][i * 128:(i + 1) * 128, :].transpose([1, 0]).bitcast(F32R))
        f1T.append(t_)
        t2 = W.tile([128, 128], F32R, tag=f"f2T{i}")
        dma.dma_start(out=t2,
                      in_=D["ff2_w"][:, i * 128:(i + 1) * 128].transpose([1, 0]).bitcast(F32R))
        f2T.append(t2)
    f1b = W.tile([128, 2], F32, tag="f1b")
    dma.dma_start(out=f1b, in_=D["ff1_b"].rearrange("(a b) -> b a", a=2))
    f2b = W.tile([128, 1], F32, tag="f2b")
    dma.dma_start(out=f2b, in_=D["ff2_b"].unsqueeze(1))

    poolw = W.tile([128, 1], F32R, tag="poolw")
    dma.dma_start(out=poolw, in_=D["pool_w"].bitcast(F32R))
    poolb = W.tile([1, 1], F32, tag="poolb")
    dma.dma_start(out=poolb, in_=D["pool_b"].unsqueeze(1))

    # cmask[p, g, n] = 1 iff 0 <= p - 15*(n - 6g) <= 14
    cmask = W.tile([90, 2, 12], F32, tag="cmask")
    nc.vector.memset(cmask, 1.0)
    nc.gpsimd.affine_select(out=cmask, in_=cmask, compare_op=ALU.is_ge,
                            fill=0.0, base=0, pattern=[[90, 2], [-15, 12]],
                            channel_multiplier=1)
    nc.gpsimd.affine_select(out=cmask, in_=cmask, compare_op=ALU.is_ge,
                            fill=0.0, base=14, pattern=[[-90, 2], [15, 12]],
                            channel_multiplier=-1)
    kcorr = float(1.0 / (15 * 15 * (T - 1)))
    fcw = W.tile([78, 64], F32, tag="fcw")
    dma.dma_start(out=fcw, in_=D["fc_w"])
    fcwk = W.tile([78, 64], F32, tag="fcwk")
    nc.vector.tensor_scalar_mul(fcwk, fcw, kcorr)
    fcb = W.tile([64, 1], F32, tag="fcb")
    dma.dma_start(out=fcb, in_=D["fc_b"].unsqueeze(1))

    fu1T = W.tile([128, 2, 128], F32, tag="fu1T")
    nc.vector.memset(fu1T[:, 1, :], 0.0)
    dma.dma_start(out=fu1T[:, 0, :], in_=D["fus1_w"][0:128, :])
    dma.dma_start(out=fu1T[0:64, 1, :], in_=D["fus1_w"][128:192, :])
    fu1b = W.tile([128, 1], F32, tag="fu1b")
    dma.dma_start(out=fu1b, in_=D["fus1_b"].unsqueeze(1))
    flg = W.tile([128, 1], F32, tag="flg")
    dma.dma_start(out=flg, in_=D["fus_ln_g"].unsqueeze(1))
    flb = W.tile([128, 1], F32, tag="flb")
    dma.dma_start(out=flb, in_=D["fus_ln_b"].unsqueeze(1))
    fu2T = W.tile([128, 64], F32, tag="fu2T")
    dma.dma_start(out=fu2T, in_=D["fus2_w"])
    fu2b = W.tile([64, 1], F32, tag="fu2b")
    dma.dma_start(out=fu2b, in_=D["fus2_b"].unsqueeze(1))

    epst = W.tile([128, 1], F32, tag="epst")
    nc.vector.memset(epst, EPS)
    dummy = W.tile([128, 1], F32, tag="dummy")

    bm_all = W.tile([12, S, 12], F32, tag="bm_all")
    fus_t = W.tile([128, S], F32, tag="fus_t")
    fus_f = W.tile([64, S], F32, tag="fus_f")

    # ================= per-sample =================
    def ln_norm(src_fm, tag, gout, bout, gelu):
        """LN over features of fm [128, T]. Returns (hat_tm, out_fm)."""
        tp = pa([TCH, 3, 128])
        for c in range(3):
            nc.tensor.transpose(tp[:, c, :], src_fm[:, c * TCH:(c + 1) * TCH],
                                ident)
        utm = sb.tile([TCH, 3, 128], F32, tag=f"{tag}_tm")
        nc.vector.tensor_copy(utm, tp)
        stats = tiny.tile([TCH, 3, 6], F32, tag=f"{tag}_st")
        for c in range(3):
            nc.vector.bn_stats(stats[:, c], utm[:, c])
        mv = tiny.tile([TCH, 3, 2], F32, tag=f"{tag}_mv")
        for c in range(3):
            nc.vector.bn_aggr(mv[:, c], stats[:, c])
        sd = tiny.tile([TCH, 3], F32, tag=f"{tag}_sd")
        nc.scalar.activation(sd, mv[:, :, 1], AF.Sqrt,
                             bias=epst[0:TCH, :], scale=1.0)
        rs = tiny.tile([TCH, 3], F32, tag=f"{tag}_rs")
        nc.vector.reciprocal(rs, sd)
        hat = sb.tile([TCH, 3, 128], F32, tag=f"{tag}_hat")
        for c in range(3):
            nc.vector.tensor_scalar(hat[:, c], utm[:, c],
                                    mv[:, c, 0:1], rs[:, c:c + 1],
                                    op0=ALU.subtract, op1=ALU.mult)
        tp2 = pa([128, 3, 128])
        for c in range(3):
            nc.tensor.transpose(tp2[:, c, 0:TCH], hat[:, c, :],
                                ident[0:TCH, 0:TCH])
        ofm = sb.tile([128, 364], F32R, tag=f"{tag}_ofm")
        nc.vector.memset(ofm.bitcast(F32)[:, 363:364], 0.0)
        nc.scalar.activation(
            ofm[:, 0:363].rearrange("a (c t) -> a c t", c=3), tp2[:, :, 0:TCH],
            AF.Gelu if gelu else AF.Identity, bias=bout, scale=gout)
        return hat, ofm

    for s in range(S):
        want_dbg = DEBUG and s < DBG_SAMPLES

        # ---- load x, transpose to feature-major ----
        xtm = sb.tile([TCH, 3, C], F32, tag="xtm")
        dma.dma_start(out=xtm, in_=D["x"][s].rearrange("(c p) f -> p c f", p=TCH))
        xfm = sb.tile([90, 2, 364], F32R, tag="xfm")
        nc.vector.memset(xfm.bitcast(F32)[:, :, 363:364], 0.0)
        for g in range(2):
            xps = pa([90, 3, 128])
            for c in range(3):
                nc.tensor.transpose(xps[:, c, 0:TCH],
                                    xtm[:, c, g * 90:(g + 1) * 90],
                                    ident[0:TCH, 0:TCH])
            nc.vector.tensor_copy(
                xfm[:, g, 0:363].rearrange("a (c t) -> a c t", c=3),
                xps[:, :, 0:TCH])
        xst = tiny.tile([90, 2, 3, 6], F32, tag="xst")
        for g in range(2):
            for c in range(3):
                nc.vector.bn_stats(
                    xst[:, g, c],
                    xfm[:, g, c * TCH:(c + 1) * TCH].bitcast(F32))
        xmv = tiny.tile([90, 2, 2], F32, tag="xmv")
        for g in range(2):
            nc.vector.bn_aggr(xmv[:, g], xst[:, g])
        if want_dbg:
            dbg(f"xfm{s}", xfm.bitcast(F32)[:, :, 0:363], (90, 726))

        # ---- proj + gelu ----
        hps = pa([96, 512])
        for g in range(2):
            nc.tensor.matmul(hps[:, 0:364], wproj[:, g, :], xfm[:, g],
                             start=(g == 0), stop=(g == 1))
        hpad = sb.tile([96, PAD + T + PAD + 5], F32R, tag="hpad")
        nc.vector.memset(hpad.bitcast(F32)[:, 0:PAD], 0.0)
        nc.vector.memset(hpad.bitcast(F32)[:, PAD + T:], 0.0)
        nc.scalar.activation(hpad[:, PAD:PAD + T], hps[:, 0:T], AF.Gelu,
                             bias=bproj, scale=1.0)
        if want_dbg:
            dbg(f"h{s}", hpad.bitcast(F32)[:, PAD:PAD + T], (96, T))

        # ---- convs (dw folded into pw), bias via ones-matmul ----
        yps = ppB.tile([96, 3, 512], F32, tag="big")
        for k in range(3):
            d = DILS[k]
            for j in range(7):
                off = PAD + (j - 3) * d
                nc.tensor.matmul(yps[:, k, 0:364], wconv[k][:, j, :],
                                 hpad[:, off:off + 364],
                                 start=(j == 0), stop=False)
            nc.tensor.matmul(yps[:, k, 0:364], cbT[:, k, :],
                             ones_row, start=False, stop=True)
        ysb = sb.tile([96, 3, T], F32, tag="ysb")
        for k in range(3):
            nc.vector.tensor_copy(ysb[:, k], yps[:, k, 0:T])
        yst = tiny.tile([96, 3, 6], F32, tag="yst")
        for k in range(3):
            nc.vector.bn_stats(yst[:, k], ysb[:, k])
        ymv = tiny.tile([96, 3, 2], F32, tag="ymv")
        for k in range(3):
            nc.vector.bn_aggr(ymv[:, k], yst[:, k])
        st6 = tiny.tile([96, 6], F32, tag="st6")
        nc.vector.tensor_copy(st6[:, 0:3], ymv[:, :, 0])
        nc.vector.tensor_tensor(st6[:, 3:6], ymv[:, :, 0], ymv[:, :, 0],
                                op=ALU.mult)
        nc.vector.tensor_tensor(st6[:, 3:6], st6[:, 3:6], ymv[:, :, 1],
                                op=ALU.add)
        gst_ps = pa([8, 6])
        nc.tensor.matmul(gst_ps, wgrp, st6, start=True, stop=True)
        gst = tiny.tile([8, 6], F32, tag="gst")
        nc.vector.tensor_copy(gst, gst_ps)
        gm2 = tiny.tile([8, 3], F32, tag="gm2")
        nc.vector.tensor_tensor(gm2, gst[:, 0:3], gst[:, 0:3], op=ALU.mult)
        gvar = tiny.tile([8, 3], F32, tag="gvar")
        nc.vector.tensor_tensor(gvar, gst[:, 3:6], gm2, op=ALU.subtract)
        nc.scalar.activation(gvar, gvar, AF.Sqrt, bias=epst[0:8, :], scale=1.0)
        nc.vector.reciprocal(gst[:, 3:6], gvar)
        bc_ps = pa([96, 6])
        nc.tensor.matmul(bc_ps, wbc, gst, start=True, stop=True)
        bc = tiny.tile([96, 6], F32, tag="bc")
        nc.vector.tensor_copy(bc, bc_ps)
        scl = tiny.tile([96, 3], F32, tag="scl")
        nc.vector.tensor_tensor(scl, gng, bc[:, 3:6], op=ALU.mult)
        bia = tiny.tile([96, 3], F32, tag="bia")
        nc.vector.tensor_tensor(bia, bc[:, 0:3], scl, op=ALU.mult)
        nc.vector.tensor_tensor(bia, gnb, bia, op=ALU.subtract)
        cat = sb.tile([96, 3, 364], F32R, tag="cat")
        nc.vector.memset(cat.bitcast(F32)[:, :, 363:364], 0.0)
        for k in range(3):
            nc.scalar.activation(cat[:, k, 0:T], ysb[:, k], AF.Gelu,
                                 bias=bia[:, k:k + 1], scale=scl[:, k:k + 1])
        if want_dbg:
            dbg(f"cat{s}", cat.bitcast(F32)[:, :, 0:T], (96, 3 * T))

        # ---- merge + LN + gelu ----
        ups = pa([128, 512])
        for g in range(3):
            nc.tensor.matmul(ups[:, 0:364], mw[g], cat[:, g],
                             start=(g == 0), stop=(g == 2))
        ufm = sb.tile([128, T], F32, tag="ufm")
        nc.scalar.activation(ufm, ups[:, 0:T], AF.Identity, bias=mb, scale=1.0)
        _, h0 = ln_norm(ufm, "mln", mlng, mlnb, gelu=True)
        if want_dbg:
            dbg(f"h0_{s}", h0.bitcast(F32)[:, 0:T], (128, T))

        # ---- qkv ----
        qps, kps, vps = pa([128, 512]), pa([128, 512]), pa([128, 512])
        nc.tensor.matmul(qps[:, 0:364], qkvT[0], h0, start=True, stop=True)
        nc.tensor.matmul(kps[:, 0:364], qkvT[1], h0, start=True, stop=True)
        nc.tensor.matmul(vps[:, 0:364], qkvT[2], h0, start=True, stop=True)
        qfm = sb.tile([64, 2, 364], F32R, tag="qfm")
        kfm = sb.tile([64, 2, 364], F32R, tag="kfm")
        nc.vector.memset(qfm.bitcast(F32)[:, :, 363:364], 0.0)
        nc.vector.memset(kfm.bitcast(F32)[:, :, 363:364], 0.0)
        for i in range(2):
            nc.scalar.activation(qfm[:, i, 0:T], qps[i * 64:(i + 1) * 64, 0:T],
                                 AF.Identity, bias=qb_s[i * 64:(i + 1) * 64, :],
                                 scale=ISQ)
            nc.vector.tensor_scalar(kfm[:, i, 0:T], kps[i * 64:(i + 1) * 64, 0:T],
                                    1.0, qb3[i * 64:(i + 1) * 64, 1:2],
                                    op0=ALU.mult, op1=ALU.add)
        vfm = sb.tile([128, T], F32, tag="vfm")
        nc.vector.tensor_scalar(vfm, vps[:, 0:T], 1.0, qb3[:, 2:3],
                                op0=ALU.mult, op1=ALU.add)
        vtp = pa([TCH, 3, 128])
        for c in range(3):
            nc.tensor.transpose(vtp[:, c, :], vfm[:, c * TCH:(c + 1) * TCH],
                                ident)
        vtm = sb.tile([TCH, 3, 4, 33], F32, tag="vtm")
        nc.vector.tensor_copy(vtm[:, :, :, 0:32],
                              vtp.rearrange("p c (h d) -> p c h d", h=4))
        nc.vector.memset(vtm[:, :, :, 32:33], 1.0)

        # ---- scores (transposed) + exp ----
        expt = sb.tile([TCH, 3, 4, T], F32, tag="expt")
        for cs in range(3):
            scps = ppB.tile([TCH, 4, 512], F32, tag="big")
            for h in range(4):
                i, b = divmod(h, 2)
                nc.tensor.matmul(
                    scps[:, h, 0:364],
                    kfm[b * 32:(b + 1) * 32, i, cs * TCH:(cs + 1) * TCH],
                    qfm[b * 32:(b + 1) * 32, i, :],
                    start=True, stop=True)
            nc.scalar.activation(expt[:, cs], scps[:, :, 0:T], AF.Exp)

        # ---- AV (token-major, Z via ones column) ----
        otm = sb.tile([TCH, 3, 4, 33], F32, tag="otm")
        for ct in range(3):
            ops_ = pa([TCH, 4, 33])
            for h in range(4):
                for cs in range(3):
                    nc.tensor.matmul(
                        ops_[:, h, :],
                        expt[:, cs, h, ct * TCH:(ct + 1) * TCH],
                        vtm[:, cs, h, :],
                        start=(cs == 0), stop=(cs == 2))
            nc.vector.tensor_copy(otm[:, ct], ops_)
        rz = tiny.tile([TCH, 3, 4], F32, tag="rz")
        nc.vector.reciprocal(rz, otm[:, :, :, 32])
        on_tm = sb.tile([TCH, 3, 128], F32, tag="on_tm")
        for ct in range(3):
            for h in range(4):
                nc.vector.tensor_scalar_mul(
                    on_tm[:, ct, h * 32:(h + 1) * 32],
                    otm[:, ct, h, 0:32], rz[:, ct, h:h + 1])
        otp = pa([128, 3, 128])
        for ct in range(3):
            nc.tensor.transpose(otp[:, ct, 0:TCH], on_tm[:, ct, :],
                                ident[0:TCH, 0:TCH])
        oat = sb.tile([128, 364], F32R, tag="oat")
        nc.vector.memset(oat.bitcast(F32)[:, 363:364], 0.0)
        nc.vector.tensor_copy(oat[:, 0:363].rearrange("a (c t) -> a c t", c=3),
                              otp[:, :, 0:TCH])
        if want_dbg:
            dbg(f"attn{s}", oat.bitcast(F32)[:, 0:T], (128, T))

        # ---- out proj + residual + LN1 ----
        rps = pa([128, 512])
        nc.tensor.matmul(rps[:, 0:364], owT, oat, start=True, stop=True)
        rfm = sb.tile([128, T], F32, tag="rfm")
        nc.scalar.activation(rfm, rps[:, 0:T], AF.Identity, bias=ob, scale=1.0)
        nc.vector.tensor_tensor(rfm, rfm, h0.bitcast(F32)[:, 0:T], op=ALU.add)
        hat1, h1 = ln_norm(rfm, "ln1", ln1g, ln1b, gelu=False)

        # ---- ffn ----
        g1 = sb.tile([128, 2, 364], F32R, tag="g1")
        nc.vector.memset(g1.bitcast(F32)[:, :, 363:364], 0.0)
        for i in range(2):
            f1ps = pa([128, 512])
            nc.tensor.matmul(f1ps[:, 0:364], f1T[i], h1,
                             start=True, stop=True)
            nc.scalar.activation(g1[:, i, 0:T], f1ps[:, 0:T], AF.Relu,
                                 bias=f1b[:, i:i + 1], scale=1.0)
        f2ps = pa([128, 512])
        for i in range(2):
            nc.tensor.matmul(f2ps[:, 0:364], f2T[i], g1[:, i],
                             start=(i == 0), stop=(i == 1))
        ffo = sb.tile([128, T], F32, tag="ffo")
        nc.scalar.activation(ffo, f2ps[:, 0:T], AF.Identity, bias=f2b, scale=1.0)
        nc.vector.tensor_tensor(ffo, ffo, h1.bitcast(F32)[:, 0:T], op=ALU.add)
        hat2, h2 = ln_norm(ffo, "ln2", ln2g, ln2b, gelu=False)
        if want_dbg:
            dbg(f"h2_{s}", h2.bitcast(F32)[:, 0:T], (128, T))

        # ---- attentive pooling ----
        plps = pa([1, 512])
        nc.tensor.matmul(plps[:, 0:364], poolw, h2, start=True, stop=True)
        pw_sb = tiny.tile([1, T], F32, tag="pw_sb")
        zp = tiny.tile([1, 1], F32, tag="zp")
        nc.scalar.activation(pw_sb, plps[:, 0:T], AF.Exp,
                             bias=poolb, scale=1.0, accum_out=zp)
        rzp = tiny.tile([1, 1], F32, tag="rzp")
        nc.vector.reciprocal(rzp, zp)
        wn = tiny.tile([1, T], F32, tag="wn")
        nc.vector.tensor_scalar_mul(wn, pw_sb, rzp)
        wtp = pa([TCH, 3, 1])
        for c in range(3):
            nc.tensor.transpose(wtp[:, c, :], wn[:, c * TCH:(c + 1) * TCH],
                                ident[0:1, 0:1])
        wcol = tiny.tile([TCH, 3, 1], F32, tag="wcol")
        nc.vector.tensor_copy(wcol, wtp)
        tps = pa([128, 1])
        for c in range(3):
            nc.tensor.matmul(tps, hat2[:, c, :], wcol[:, c, :],
                             start=(c == 0), stop=(c == 2))
        nc.vector.tensor_scalar(fus_t[:, s:s + 1], tps, ln2g, ln2b,
                                op0=ALU.mult, op1=ALU.add)

        # ---- correlation fingerprint ----
        cstd = tiny.tile([90, 2], F32, tag="cstd")
        nc.scalar.activation(cstd, xmv[:, :, 1], AF.Sqrt, bias=0.0,
                             scale=float(T) / (T - 1))
        nc.vector.tensor_scalar_max(cstd, cstd, 1e-8)
        cinv = tiny.tile([90, 2], F32, tag="cinv")
        nc.vector.reciprocal(cinv, cstd)
        wcorr = sb.tile([90, 2, 12], F32R, tag="wcorr")
        for g in range(2):
            nc.vector.tensor_scalar_mul(wcorr[:, g], cmask[:, g],
                                        cinv[:, g:g + 1])
        swps = pa([12, 512])
        for g in range(2):
            nc.tensor.matmul(swps[:, 0:364], wcorr[:, g], xfm[:, g],
                             start=(g == 0), stop=(g == 1))
        swsb = tiny.tile([12, T], F32, tag="swsb")
        rsum = tiny.tile([12, 1], F32, tag="rsum")
        nc.vector.tensor_scalar(swsb, swps[:, 0:T], 1.0, 0.0, op0=ALU.mult,
                                op1=ALU.add, accum_out=rsum)
        swtp = pa([TCH, 3, 12])
        for c in range(3):
            nc.tensor.transpose(swtp[:, c, :], swsb[:, c * TCH:(c + 1) * TCH],
                                ident[0:12, 0:12])
        swtm = tiny.tile([TCH, 3, 12], F32, tag="swtm")
        nc.vector.tensor_copy(swtm, swtp)
        rsT_ps = pa([1, 12])
        nc.tensor.transpose(rsT_ps, rsum, ident[0:12, 0:12])
        rsT = tiny.tile([1, 12], F32, tag="rsT")
        nc.vector.tensor_copy(rsT, rsT_ps)
        rsTn = tiny.tile([1, 12], F32, tag="rsTn")
        nc.vector.tensor_scalar_mul(rsTn, rsT, -1.0 / T)
        gps = pa([12, 12])
        for c in range(3):
            nc.tensor.matmul(gps, swtm[:, c, :], swtm[:, c, :],
                             start=(c == 0), stop=False)
        nc.tensor.matmul(gps, rsTn, rsT, start=False, stop=True)
        nc.vector.tensor_copy(bm_all[:, s, :], gps)

    # ================= batched tail =================
    bm_dram = nc.dram_tensor("bm_scratch", [12, S, 12], F32).ap()
    dma.dma_start(out=bm_dram, in_=bm_all)
    fcv = W.tile([78, S], F32, tag="fcv")
    row_off = 0
    for i in range(12):
        n = 12 - i
        dma.dma_start(
            out=fcv[row_off:row_off + n, :],
            in_=bm_dram[i, :, i:12].transpose([1, 0]))
        row_off += n
    fcps = pa([64, S])
    nc.tensor.matmul(fcps, fcwk, fcv, start=True, stop=True)
    nc.scalar.activation(fus_f, fcps, AF.Gelu, bias=fcb, scale=1.0)

    fu_ps = pa([128, S])
    nc.tensor.matmul(fu_ps, fu1T[:, 0, :], fus_t, start=True, stop=False)
    nc.tensor.matmul(fu_ps, fu1T[0:64, 1, :], fus_f, start=False, stop=True)
    zfm = W.tile([128, S], F32, tag="zfm")
    nc.vector.tensor_scalar(zfm, fu_ps, 1.0, fu1b, op0=ALU.mult, op1=ALU.add)
    ztp = pa([S, 128])
    nc.tensor.transpose(ztp, zfm, ident)
    ztm = W.tile([S, 128], F32, tag="ztm")
    nc.vector.tensor_copy(ztm, ztp)
    zst = W.tile([S, 6], F32, tag="zst")
    nc.vector.bn_stats(zst, ztm)
    zmv = W.tile([S, 2], F32, tag="zmv")
    nc.vector.bn_aggr(zmv, zst)
    zsd = W.tile([S, 1], F32, tag="zsd")
    nc.scalar.activation(zsd, zmv[:, 1:2], AF.Sqrt, bias=epst[0:S, :], scale=1.0)
    zrs = W.tile([S, 1], F32, tag="zrs")
    nc.vector.reciprocal(zrs, zsd)
    zhat = W.tile([S, 128], F32, tag="zhat")
    nc.vector.tensor_scalar(zhat, ztm, zmv[:, 0:1], zrs,
                            op0=ALU.subtract, op1=ALU.mult)
    zhtp = pa([128, S])
    nc.tensor.transpose(zhtp, zhat, ident[0:S, 0:S])
    zg = W.tile([128, S], F32, tag="zg")
    nc.scalar.activation(zg, zhtp, AF.Gelu, bias=flb, scale=flg)
    out_ps = pa([64, S])
    nc.tensor.matmul(out_ps, fu2T, zg, start=True, stop=True)
    out_sb = W.tile([64, S], F32, tag="out_sb")
    nc.scalar.activation(out_sb, out_ps, AF.Identity, bias=fu2b, scale=1.0)
    outT_ps = pa([S, 64])
    nc.tensor.transpose(outT_ps, out_sb, ident[0:64, 0:64])
    outT = W.tile([S, 64], F32, tag="outT")
    nc.vector.tensor_copy(outT, outT_ps)
    dma.dma_start(out=out_dram, in_=outT)

    for p in reversed(pools):
        p.__exit__(None, None, None)


_PROGRAM = None


def _get_program():
    global _PROGRAM
    if _PROGRAM is None:
        _PROGRAM = build_program()
    return _PROGRAM


def kernel(**inputs):
    from concourse.bass_utils import run_bass_kernel_spmd

    nc, _ = _get_program()
    in_maps = []
    for c in range(NCORES):
        m = {}
        for name, _shape in INPUT_SPECS:
            if name == "x":
                m["x"] = np.ascontiguousarray(
                    np.asarray(inputs["x"][c * S:(c + 1) * S], dtype=np.float32))
            else:
                m[name] = np.ascontiguousarray(
                    np.asarray(inputs[name], dtype=np.float32))
        in_maps.append(m)
    res = run_bass_kernel_spmd(nc, in_maps, list(range(NCORES)))
    global LAST_RESULTS
    LAST_RESULTS = res
    out = np.concatenate([res.results[c]["out"] for c in range(NCORES)], axis=0)
    return out.astype(np.float32)


LAST_RESULTS = None



# revision 19
# speedup vs baseline: 1.1890x; 1.1890x over previous
"""Trainium2 Bass kernel for nn_NetworkAwareClassicalExpert (B=256,T=363,C=180).

Data-parallel over 8 NeuronCores: 32 samples/core. Per-core program processes
samples in PAIRS with stage-level interleaving so engines overlap across the
two in-flight samples.

vs the original single-sample version:
  - all large matmuls and transposes in bf16 (fp32 matmuls/transposes lower
    to 2 HW matmuls each; bf16 streams 1 col/cycle at any size). Stats, LN
    scalar math, and the tiny GroupNorm aggregation matmuls stay fp32.
  - attention AV computed s-major (P as moving rhs): 12 matmuls with the
    softmax normalizer Z from a ones-column in V; per-head reciprocal +
    mask-matmul broadcast replaces token-major normalization + transposes
  - conv bias folded into the GroupNorm affine (no per-sample bias matmuls)
  - residual adds folded into PSUM via identity matmuls
  - per-sample rsqrt via bit-hack + Newton iterations (DVE init, GpSimd
    iterations) so the Scalar engine only loads the Gelu and Exp activation
    tables (2 loads per sample pair, vs ~8 table loads per sample)
"""

import sys
import os

sys.path.insert(0, "/opt/trn_rl_repo")

import numpy as np

import concourse.bass as bass
import concourse.mybir as mybir
import concourse.tile as tile
import bass_rust
from concourse.vector_clock import ScopedClock
from concourse.masks import make_identity

F32 = mybir.dt.float32
BF16 = mybir.dt.bfloat16
I32 = mybir.dt.int32
AF = mybir.ActivationFunctionType
ALU = mybir.AluOpType

B, T, C = 256, 363, 180
CD = 96
DM = 128
DILS = (1, 4, 16)
NCORES = 8
S = int(os.environ.get("KB_NSAMP", str(B // NCORES)))
TCH = 121                # t-chunk (3 chunks of 121)
PAD = 48
EPS = 1e-5
ISQ = float(1.0 / np.sqrt(32.0))
MAGIC = 0x5F3759DF

DEBUG = bool(int(os.environ.get("KBDBG", "0")))
DBG_SAMPLES = int(os.environ.get("KBDBG_S", "2"))


def _patch_tile_drain():
    """This walrus rejects >1 sem wait on the final Tile drain: split them."""

    def _drain_and_barrier(self, tick_clock, wait_clock):
        drain_inst = self.nc.sync.drain()
        wait_clock.add_sem_waits(
            drain_inst.ins, ScopedClock({None: tick_clock.global_clock})
        )
        si = drain_inst.ins.sync_info
        if si is not None and si.on_wait is not None and len(si.on_wait) > 1:
            waits = list(si.on_wait)
            ups = list(si.on_update) if si.on_update else []
            drain_inst.ins.sync_info = bass_rust.SyncInfo(
                on_wait=waits[:1], on_update=ups
            )
            for w in waits[1:]:
                nop = self.nc.sync.nop()
                nop.ins.sync_info = bass_rust.SyncInfo(on_wait=[w], on_update=[])
        self.nc.all_engine_barrier()
        popped = self.nc._tile_sem_poison_stack.pop()
        assert popped is self._sem_poison
        if not int(os.environ.get("KB_NOSEMCLEAR", "0")):
            self.nc.clear_and_free_semaphores(list(self.sems.allocated().values()))
        self.nc.all_engine_barrier()

    tile.TileContext._drain_and_barrier = _drain_and_barrier


_patch_tile_drain()


def _split_sync_waits(nc, max_waits=1):
    """walrus rejects instructions with >1 sem wait; hoist excess onto
    same-engine NOPs inserted immediately before (created via the real
    bass emitters so they carry all rust-side init)."""
    for f in nc.m.functions:
        for bb in f.blocks:
            insts = list(bb.instructions)
            out = []
            changed = False
            for inst in insts:
                si = getattr(inst, "sync_info", None)
                if si is not None and si.on_wait and len(si.on_wait) > max_waits:
                    waits = list(si.on_wait)
                    ups = list(si.on_update) if si.on_update else []
                    extra = waits[max_waits:]
                    for i in range(0, len(extra), max_waits):
                        nop = nc.engines[inst.engine].nop(nofuse=True)
                        cur = nn_cur_bb(nc)
                        lst = list(cur.instructions)
                        assert lst and lst[-1].name == nop.ins.name
                        cur.instructions = lst[:-1]
                        nop.ins.sync_info = bass_rust.SyncInfo(
                            on_wait=extra[i:i + max_waits], on_update=[])
                        out.append(nop.ins)
                    inst.sync_info = bass_rust.SyncInfo(
                        on_wait=waits[:max_waits], on_update=ups)
                    changed = True
                out.append(inst)
            if changed:
                bb.instructions = out


def nn_cur_bb(nc):
    bbw = nc.cur_bb
    return bbw.bb if hasattr(bbw, "bb") else bbw


INPUT_SPECS = [
    ("x", (S, T, C)),
    ("w_proj", (12, 15, 8)), ("b_proj", (12, 8)),
    ("dw_w", (3, 96, 7)), ("dw_b", (3, 96)),
    ("pw_w", (3, 96, 96)), ("pw_b", (3, 96)),
    ("gn_g", (3, 96)), ("gn_b", (3, 96)),
    ("merge_w", (288, 128)), ("merge_b", (128,)),
    ("merge_ln_g", (128,)), ("merge_ln_b", (128,)),
    ("qkv_w", (384, 128)), ("qkv_b", (384,)),
    ("out_w", (128, 128)), ("out_b", (128,)),
    ("ln1_g", (128,)), ("ln1_b", (128,)),
    ("ff1_w", (256, 128)), ("ff1_b", (256,)),
    ("ff2_w", (128, 256)), ("ff2_b", (128,)),
    ("ln2_g", (128,)), ("ln2_b", (128,)),
    ("pool_w", (128, 1)), ("pool_b", (1,)),
    ("fc_w", (78, 64)), ("fc_b", (64,)),
    ("fus1_w", (192, 128)), ("fus1_b", (128,)),
    ("fus_ln_g", (128,)), ("fus_ln_b", (128,)),
    ("fus2_w", (128, 64)), ("fus2_b", (64,)),
]


def build_program():
    nc = bass.Bass("TRN2", target_bir_lowering=False, debug=False,
                   num_devices=NCORES)
    D = {}
    for name, shape in INPUT_SPECS:
        D[name] = nc.dram_tensor(name, list(shape), F32, kind="ExternalInput").ap()
    out_dram = nc.dram_tensor("out", [S, 64], F32, kind="ExternalOutput").ap()

    with tile.TileContext(nc) as tc:
        _build(nc, tc, D, out_dram)
    if not int(os.environ.get("KB_NOSPLIT", "0")):
        _split_sync_waits(nc)
    return nc, {}


def _build(nc, tc, D, out_dram):
    pools = []

    def mkpool(name, bufs, space="SBUF"):
        p = tc.tile_pool(name=name, bufs=bufs, space=space)
        pools.append(p)
        return p.__enter__()

    W = mkpool("weights", 1)        # persistent tiles, one tag each
    early = mkpool("early", 3)      # front-of-pipe tiles (next-pair prefetch)
    sb = mkpool("sb", 2)            # per-sample activations
    tiny = mkpool("tiny", 2)        # small per-sample stats
    Psc = mkpool("psc", 1, "PSUM")  # scores [121,2,512] -> 2 banks
    Pcv = mkpool("pcv", 2, "PSUM")  # conv/proj [96,512]  -> 2 banks
    Pm = mkpool("pm", 4, "PSUM")    # everything else [128,512] -> 4 banks

    dma = nc.sync

    _pm_n = [0]

    def pm(shape, dtype=F32):
        _pm_n[0] += 1
        return Pm.tile(list(shape), dtype, tag="pm", name=f"pm{_pm_n[0]}")

    def rsqrt_newton(y, v, sc, n_iter=2):
        """y <- rsqrt(v) elementwise. y, v, sc float32 SBUF tiles, same shape.
        Bit-hack init on DVE, Newton iterations on GpSimd."""
        yb = y.bitcast(I32)
        vb = v.bitcast(I32)
        nc.vector.tensor_scalar(yb, vb, 1, -1, op0=ALU.logical_shift_right,
                                op1=ALU.bitwise_xor)
        nc.vector.tensor_scalar(yb, yb, MAGIC + 1, None, op0=ALU.add)
        for _ in range(n_iter):
            nc.gpsimd.tensor_tensor(sc, y, y, op=ALU.mult)
            nc.gpsimd.tensor_tensor(sc, sc, v, op=ALU.mult)
            nc.gpsimd.tensor_single_scalar(sc, sc, -0.5, op=ALU.mult)
            nc.gpsimd.tensor_single_scalar(sc, sc, 1.5, op=ALU.add)
            nc.gpsimd.tensor_tensor(y, y, sc, op=ALU.mult)

    def bcast(name, src):
        """Cast an f32 W-tile to a bf16 W-tile."""
        t = W.tile(list(src.shape), BF16, tag=f"{name}_b", name=f"{name}_b")
        nc.vector.tensor_copy(t, src)
        return t

    # ================= weight preload =================
    ident = W.tile([128, 128], F32, tag="ident")
    make_identity(nc, ident)
    identb = bcast("ident", ident)

    wproj_f = W.tile([90, 2, 96], F32, tag="wproj_f")
    nc.vector.memset(wproj_f, 0.0)
    for n in range(12):
        g, j = divmod(n, 6)
        dma.dma_start(out=wproj_f[j * 15:(j + 1) * 15, g, n * 8:(n + 1) * 8],
                      in_=D["w_proj"][n])
    wproj = bcast("wproj", wproj_f)
    bproj = W.tile([96, 1], F32, tag="bproj")
    dma.dma_start(out=bproj, in_=D["b_proj"].rearrange("a b -> (a b)").unsqueeze(1))

    pwT, dwk = [], []
    for k in range(3):
        t_ = W.tile([96, 96], F32, tag=f"pwT{k}")
        dma.dma_start(out=t_, in_=D["pw_w"][k].transpose([1, 0]))
        pwT.append(t_)
        t2 = W.tile([96, 7], F32, tag=f"dw{k}")
        dma.dma_start(out=t2, in_=D["dw_w"][k])
        dwk.append(t2)
    wconv = []
    for k in range(3):
        t_ = W.tile([96, 7, 96], BF16, tag=f"wconv{k}")
        for j in range(7):
            nc.vector.tensor_scalar_mul(t_[:, j, :], pwT[k], dwk[k][:, j:j + 1])
        wconv.append(t_)
    dwb = W.tile([96, 3], F32, tag="dwb")
    dma.dma_start(out=dwb, in_=D["dw_b"].transpose([1, 0]))
    pwb = W.tile([96, 3], F32, tag="pwb")
    dma.dma_start(out=pwb, in_=D["pw_b"].transpose([1, 0]))
    cb_ps = pm([96, 3])
    for k in range(3):
        nc.tensor.matmul(cb_ps[:, k:k + 1], pwT[k], dwb[:, k:k + 1],
                         start=True, stop=True, skip_group_check=True)
    cb = W.tile([96, 3], F32, tag="cb")
    nc.vector.tensor_add(cb, cb_ps, pwb)

    gng = W.tile([96, 3], F32, tag="gng")
    dma.dma_start(out=gng, in_=D["gn_g"].transpose([1, 0]))
    gnb = W.tile([96, 3], F32, tag="gnb")
    dma.dma_start(out=gnb, in_=D["gn_b"].transpose([1, 0]))

    # wgrp[c, g] = 1/12 iff 0 <= c - 12g <= 11 ; wbc[g, c] = 1 iff same
    wgrp = W.tile([96, 8], F32, tag="wgrp")
    nc.vector.memset(wgrp, 1.0 / 12.0)
    nc.gpsimd.affine_select(out=wgrp, in_=wgrp, compare_op=ALU.is_ge,
                            fill=0.0, base=0, pattern=[[-12, 8]],
                            channel_multiplier=1)
    nc.gpsimd.affine_select(out=wgrp, in_=wgrp, compare_op=ALU.is_ge,
                            fill=0.0, base=11, pattern=[[12, 8]],
                            channel_multiplier=-1)
    wbc = W.tile([8, 96], F32, tag="wbc")
    nc.vector.memset(wbc, 1.0)
    nc.gpsimd.affine_select(out=wbc, in_=wbc, compare_op=ALU.is_ge,
                            fill=0.0, base=0, pattern=[[1, 96]],
                            channel_multiplier=-12)
    nc.gpsimd.affine_select(out=wbc, in_=wbc, compare_op=ALU.is_ge,
                            fill=0.0, base=11, pattern=[[-1, 96]],
                            channel_multiplier=12)

    # hmask[0, i, p] = 1 iff 64i <= p <= 64i+32  (head band incl Z row)
    hmask_f = W.tile([1, 2, 128], F32, tag="hmask_f")
    nc.vector.memset(hmask_f, 1.0)
    nc.gpsimd.affine_select(out=hmask_f, in_=hmask_f, compare_op=ALU.is_ge,
                            fill=0.0, base=0, pattern=[[-64, 2], [1, 128]],
                            channel_multiplier=0)
    nc.gpsimd.affine_select(out=hmask_f, in_=hmask_f, compare_op=ALU.is_ge,
                            fill=0.0, base=32, pattern=[[64, 2], [-1, 128]],
                            channel_multiplier=0)
    hmask = bcast("hmask", hmask_f)

    mw_f = W.tile([96, 3, 128], F32, tag="mw_f")
    for g in range(3):
        dma.dma_start(out=mw_f[:, g, :], in_=D["merge_w"][g * 96:(g + 1) * 96, :])
    mw = bcast("mw", mw_f)
    mb = W.tile([128, 1], F32, tag="mb")
    dma.dma_start(out=mb, in_=D["merge_b"].unsqueeze(1))
    mlng = W.tile([128, 1], F32, tag="mlng")
    dma.dma_start(out=mlng, in_=D["merge_ln_g"].unsqueeze(1))
    mlnb = W.tile([128, 1], F32, tag="mlnb")
    dma.dma_start(out=mlnb, in_=D["merge_ln_b"].unsqueeze(1))

    qkvT_f = W.tile([128, 3, 128], F32, tag="qkvT_f")
    for i in range(3):
        dma.dma_start(out=qkvT_f[:, i, :],
                      in_=D["qkv_w"][i * 128:(i + 1) * 128, :].transpose([1, 0]))
    qkvT = bcast("qkvT", qkvT_f)
    qb3 = W.tile([128, 3], F32, tag="qb3")
    dma.dma_start(out=qb3, in_=D["qkv_b"].rearrange("(a b) -> b a", a=3))
    qb_s = W.tile([128, 1], F32, tag="qb_s")
    nc.vector.tensor_scalar_mul(qb_s, qb3[:, 0:1], ISQ)

    owT_f = W.tile([128, 128], F32, tag="owT_f")
    dma.dma_start(out=owT_f, in_=D["out_w"].transpose([1, 0]))
    owT = bcast("owT", owT_f)
    ob = W.tile([128, 1], F32, tag="ob")
    dma.dma_start(out=ob, in_=D["out_b"].unsqueeze(1))

    ln1g = W.tile([128, 1], F32, tag="ln1g")
    dma.dma_start(out=ln1g, in_=D["ln1_g"].unsqueeze(1))
    ln1b = W.tile([128, 1], F32, tag="ln1b")
    dma.dma_start(out=ln1b, in_=D["ln1_b"].unsqueeze(1))
    ln2g = W.tile([128, 1], F32, tag="ln2g")
    dma.dma_start(out=ln2g, in_=D["ln2_g"].unsqueeze(1))
    ln2b = W.tile([128, 1], F32, tag="ln2b")
    dma.dma_start(out=ln2b, in_=D["ln2_b"].unsqueeze(1))

    f12_f = W.tile([128, 4, 128], F32, tag="f12_f")
    for i in range(2):
        dma.dma_start(out=f12_f[:, i, :],
                      in_=D["ff1_w"][i * 128:(i + 1) * 128, :].transpose([1, 0]))
        dma.dma_start(out=f12_f[:, 2 + i, :],
                      in_=D["ff2_w"][:, i * 128:(i + 1) * 128].transpose([1, 0]))
    f12 = bcast("f12", f12_f)
    f1T = [f12[:, 0, :], f12[:, 1, :]]
    f2T = [f12[:, 2, :], f12[:, 3, :]]
    f1b = W.tile([128, 2], F32, tag="f1b")
    dma.dma_start(out=f1b, in_=D["ff1_b"].rearrange("(a b) -> b a", a=2))
    f2b = W.tile([128, 1], F32, tag="f2b")
    dma.dma_start(out=f2b, in_=D["ff2_b"].unsqueeze(1))

    poolw_f = W.tile([128, 1], F32, tag="poolw_f")
    dma.dma_start(out=poolw_f, in_=D["pool_w"])
    poolw = bcast("poolw", poolw_f)
    poolb = W.tile([1, 1], F32, tag="poolb")
    dma.dma_start(out=poolb, in_=D["pool_b"].unsqueeze(1))

    # cmask[p, g, n] = 1 iff 0 <= p - 15*(n - 6g) <= 14
    cmask = W.tile([90, 2, 12], F32, tag="cmask")
    nc.vector.memset(cmask, 1.0)
    nc.gpsimd.affine_select(out=cmask, in_=cmask, compare_op=ALU.is_ge,
                            fill=0.0, base=0, pattern=[[90, 2], [-15, 12]],
                            channel_multiplier=1)
    nc.gpsimd.affine_select(out=cmask, in_=cmask, compare_op=ALU.is_ge,
                            fill=0.0, base=14, pattern=[[-90, 2], [15, 12]],
                            channel_multiplier=-1)
    kcorr = float(1.0 / (15 * 15 * (T - 1)))
    fcw = W.tile([78, 64], F32, tag="fcw")
    dma.dma_start(out=fcw, in_=D["fc_w"])
    fcwk = W.tile([78, 64], BF16, tag="fcwk")
    nc.vector.tensor_scalar_mul(fcwk, fcw, kcorr)
    fcb = W.tile([64, 1], F32, tag="fcb")
    dma.dma_start(out=fcb, in_=D["fc_b"].unsqueeze(1))

    fu1T_f = W.tile([128, 2, 128], F32, tag="fu1T_f")
    nc.vector.memset(fu1T_f[:, 1, :], 0.0)
    dma.dma_start(out=fu1T_f[:, 0, :], in_=D["fus1_w"][0:128, :])
    dma.dma_start(out=fu1T_f[0:64, 1, :], in_=D["fus1_w"][128:192, :])
    fu1T = bcast("fu1T", fu1T_f)
    fu1b = W.tile([128, 1], F32, tag="fu1b")
    dma.dma_start(out=fu1b, in_=D["fus1_b"].unsqueeze(1))
    flg = W.tile([128, 1], F32, tag="flg")
    dma.dma_start(out=flg, in_=D["fus_ln_g"].unsqueeze(1))
    flb = W.tile([128, 1], F32, tag="flb")
    dma.dma_start(out=flb, in_=D["fus_ln_b"].unsqueeze(1))
    fu2T_f = W.tile([128, 64], F32, tag="fu2T_f")
    dma.dma_start(out=fu2T_f, in_=D["fus2_w"])
    fu2T = bcast("fu2T", fu2T_f)
    fu2b = W.tile([64, 1], F32, tag="fu2b")
    dma.dma_start(out=fu2b, in_=D["fus2_b"].unsqueeze(1))

    bm_all = W.tile([12, S, 12], F32, tag="bm_all")
    fus_t = W.tile([128, S], BF16, tag="fus_t")
    fus_f = W.tile([64, S], BF16, tag="fus_f")

    # ================= per-sample stages =================

    def ln_norm(cx, src_fm, tag, gout, bout, gelu):
        """LN over features of src_fm [128, T] (bf16). -> (hat_tm, out_fm)."""
        tp = pm([TCH, 3, 128], BF16)
        for c in range(3):
            nc.tensor.transpose(tp[:, c, :],
                                src_fm[:, c * TCH:(c + 1) * TCH], identb)
        utm = sb.tile([TCH, 3, 128], F32, tag=f"{tag}_tm")
        nc.vector.tensor_copy(utm, tp)
        stats = tiny.tile([TCH, 3, 6], F32, tag=f"{tag}_st")
        for c in range(3):
            nc.vector.bn_stats(stats[:, c], utm[:, c])
        mv = tiny.tile([TCH, 3, 2], F32, tag=f"{tag}_mv")
        for c in range(3):
            nc.vector.bn_aggr(mv[:, c], stats[:, c])
        vpe = tiny.tile([TCH, 3], F32, tag=f"{tag}_vpe")
        nc.gpsimd.tensor_single_scalar(vpe, mv[:, :, 1], EPS, op=ALU.add)
        rs = tiny.tile([TCH, 3], F32, tag=f"{tag}_rs")
        nsc = tiny.tile([TCH, 3], F32, tag=f"{tag}_nsc")
        rsqrt_newton(rs, vpe, nsc)
        hat = sb.tile([TCH, 3, 128], BF16, tag=f"{tag}_hat")
        for c in range(3):
            nc.vector.tensor_scalar(hat[:, c], utm[:, c],
                                    mv[:, c, 0:1], rs[:, c:c + 1],
                                    op0=ALU.subtract, op1=ALU.mult)
        tp2 = pm([128, 3, 128], BF16)
        for c in range(3):
            nc.tensor.transpose(tp2[:, c, 0:TCH], hat[:, c, :],
                                identb[0:TCH, 0:TCH])
        ofm = sb.tile([128, T], BF16, tag=f"{tag}_ofm")
        nc.scalar.activation(
            ofm.rearrange("a (c t) -> a c t", c=3), tp2[:, :, 0:TCH],
            AF.Gelu if gelu else AF.Identity, bias=bout, scale=gout)
        return hat, ofm

    def st_ld(s, cx):
        xtm = early.tile([TCH, 3, C], F32, tag="xtm")
        dma.dma_start(out=xtm, in_=D["x"][s].rearrange("(c p) f -> p c f", p=TCH))
        xtb = early.tile([TCH, 3, C], BF16, tag="xtb")
        nc.vector.tensor_copy(xtb, xtm)
        xfm = early.tile([90, 2, T], BF16, tag="xfm")
        for g in range(2):
            xps = pm([90, 3, 128], BF16)
            for c in range(3):
                nc.tensor.transpose(xps[:, c, 0:TCH],
                                    xtb[:, c, g * 90:(g + 1) * 90],
                                    identb[0:TCH, 0:TCH])
            nc.vector.tensor_copy(
                xfm[:, g, :].rearrange("a (c t) -> a c t", c=3),
                xps[:, :, 0:TCH])
        xst = tiny.tile([90, 2, 3, 6], F32, tag="xst")
        for g in range(2):
            for c in range(3):
                nc.vector.bn_stats(
                    xst[:, g, c], xfm[:, g, c * TCH:(c + 1) * TCH])
        xmv = tiny.tile([90, 2, 2], F32, tag="xmv")
        for g in range(2):
            nc.vector.bn_aggr(xmv[:, g], xst[:, g])
        cx["xfm"] = xfm
        cx["xmv"] = xmv

    def st_proj(s, cx):
        hps = Pcv.tile([96, 512], F32, tag="cv", name="hps")
        for g in range(2):
            nc.tensor.matmul(hps[:, 0:T], wproj[:, g, :], cx["xfm"][:, g, :],
                             start=(g == 0), stop=(g == 1))
        hpad = early.tile([96, PAD + T + PAD + 5], BF16, tag="hpad")
        nc.gpsimd.memset(hpad[:, 0:PAD], 0.0)
        nc.gpsimd.memset(hpad[:, PAD + T:], 0.0)
        nc.scalar.activation(hpad[:, PAD:PAD + T], hps[:, 0:T], AF.Gelu,
                             bias=bproj, scale=1.0)
        cx["hpad"] = hpad

    def st_conv(s, cx):
        ysb = sb.tile([96, 3, T], F32, tag="ysb")
        hpad = cx["hpad"]
        for k in range(3):
            d = DILS[k]
            yps = Pcv.tile([96, 512], F32, tag="cv", name=f"yps{k}")
            for j in range(7):
                off = PAD + (j - 3) * d
                nc.tensor.matmul(yps[:, 0:364], wconv[k][:, j, :],
                                 hpad[:, off:off + 364],
                                 start=(j == 0), stop=(j == 6))
            nc.vector.tensor_copy(ysb[:, k], yps[:, 0:T])
        yst = tiny.tile([96, 3, 6], F32, tag="yst")
        for k in range(3):
            nc.vector.bn_stats(yst[:, k], ysb[:, k])
        ymv = tiny.tile([96, 3, 2], F32, tag="ymv")
        for k in range(3):
            nc.vector.bn_aggr(ymv[:, k], yst[:, k])
        cx["ysb"] = ysb
        cx["ymv"] = ymv

    def st_gn(s, cx):
        ymv = cx["ymv"]
        st6 = tiny.tile([96, 6], F32, tag="st6")
        # true per-channel mean m' = m_nobias + cb ; E[y^2] = m'^2 + var
        nc.gpsimd.tensor_tensor(st6[:, 0:3], ymv[:, :, 0], cb, op=ALU.add)
        nc.gpsimd.tensor_tensor(st6[:, 3:6], st6[:, 0:3], st6[:, 0:3],
                                op=ALU.mult)
        nc.gpsimd.tensor_tensor(st6[:, 3:6], st6[:, 3:6], ymv[:, :, 1],
                                op=ALU.add)
        gst_ps = pm([8, 6])
        nc.tensor.matmul(gst_ps, wgrp, st6, start=True, stop=True)
        gst = tiny.tile([8, 6], F32, tag="gst")
        nc.vector.tensor_copy(gst, gst_ps)
        gvar = tiny.tile([8, 3], F32, tag="gvar")
        # var_g + eps = E[y^2]_g - m_g^2 + eps
        nc.gpsimd.tensor_tensor(gvar, gst[:, 0:3], gst[:, 0:3], op=ALU.mult)
        nc.gpsimd.tensor_tensor(gvar, gst[:, 3:6], gvar, op=ALU.subtract)
        nc.gpsimd.tensor_single_scalar(gvar, gvar, EPS, op=ALU.add)
        gsc = tiny.tile([8, 3], F32, tag="gv_nsc")
        rsqrt_newton(gst[:, 3:6], gvar, gsc)
        bc_ps = pm([96, 6])
        nc.tensor.matmul(bc_ps, wbc, gst, start=True, stop=True)
        scl = tiny.tile([96, 3], F32, tag="scl")
        nc.vector.tensor_tensor(scl, gng, bc_ps[:, 3:6], op=ALU.mult)
        bia = tiny.tile([96, 3], F32, tag="bia")
        # bia = gnb + (cb - gm) * scl
        nc.vector.tensor_tensor(bia, cb, bc_ps[:, 0:3], op=ALU.subtract)
        nc.gpsimd.tensor_tensor(bia, bia, scl, op=ALU.mult)
        nc.gpsimd.tensor_tensor(bia, bia, gnb, op=ALU.add)
        cat = sb.tile([96, 3, T], BF16, tag="cat")
        for k in range(3):
            nc.scalar.activation(cat[:, k], cx["ysb"][:, k], AF.Gelu,
                                 bias=bia[:, k:k + 1], scale=scl[:, k:k + 1])
        cx["cat"] = cat

    def st_merge(s, cx):
        ups = pm([128, 512])
        for g in range(3):
            nc.tensor.matmul(ups[:, 0:T], mw[:, g, :], cx["cat"][:, g],
                             start=(g == 0), stop=(g == 2))
        ufm = sb.tile([128, T], BF16, tag="ufm")
        nc.scalar.activation(ufm, ups[:, 0:T], AF.Identity, bias=mb, scale=1.0)
        _, h0 = ln_norm(cx, ufm, "mln", mlng, mlnb, gelu=True)
        cx["h0"] = h0

    def st_qkv(s, cx):
        h0 = cx["h0"]
        qps, kps, vps = pm([128, 512]), pm([128, 512]), pm([128, 512])
        nc.tensor.matmul(qps[:, 0:T], qkvT[:, 0, :], h0, start=True, stop=True)
        nc.tensor.matmul(kps[:, 0:T], qkvT[:, 1, :], h0, start=True, stop=True)
        nc.tensor.matmul(vps[:, 0:T], qkvT[:, 2, :], h0, start=True, stop=True)
        qfm = sb.tile([64, 2, T], BF16, tag="qfm")
        kfm = sb.tile([64, 2, T], BF16, tag="kfm")
        for i in range(2):
            nc.scalar.activation(qfm[:, i, :], qps[i * 64:(i + 1) * 64, 0:T],
                                 AF.Identity, bias=qb_s[i * 64:(i + 1) * 64, :],
                                 scale=ISQ)
            nc.vector.tensor_scalar(kfm[:, i, :], kps[i * 64:(i + 1) * 64, 0:T],
                                    1.0, qb3[i * 64:(i + 1) * 64, 1:2],
                                    op0=ALU.mult, op1=ALU.add)
        vfm = sb.tile([128, T], BF16, tag="vfm")
        nc.vector.tensor_scalar(vfm, vps[:, 0:T], 1.0, qb3[:, 2:3],
                                op0=ALU.mult, op1=ALU.add)
        vtp = pm([TCH, 3, 128], BF16)
        for c in range(3):
            nc.tensor.transpose(vtp[:, c, :],
                                vfm[:, c * TCH:(c + 1) * TCH], identb)
        vtm = sb.tile([TCH, 3, 4, 33], BF16, tag="vtm")
        nc.vector.tensor_copy(vtm[:, :, :, 0:32],
                              vtp.rearrange("p c (h d) -> p c h d", h=4))
        nc.gpsimd.memset(vtm[:, :, :, 32:33], 1.0)
        cx["qfm"] = qfm
        cx["kfm"] = kfm
        cx["vtm"] = vtm

    def st_scores(s, cx):
        qfm, kfm = cx["qfm"], cx["kfm"]
        expt = sb.tile([TCH, 3, 4, T], BF16, tag="expt")
        for cs in range(3):
            for hp in range(2):
                scps = Psc.tile([TCH, 2, 512], F32, tag="sc", name="scps")
                for hh in range(2):
                    h = 2 * hp + hh
                    i, b = divmod(h, 2)
                    nc.tensor.matmul(
                        scps[:, hh, 0:T],
                        kfm[b * 32:(b + 1) * 32, i, cs * TCH:(cs + 1) * TCH],
                        qfm[b * 32:(b + 1) * 32, i, :],
                        start=True, stop=True)
                nc.scalar.activation(expt[:, cs, 2 * hp:2 * hp + 2, :],
                                     scps[:, :, 0:T], AF.Exp)
        cx["expt"] = expt

    def st_av(s, cx):
        expt, vtm = cx["expt"], cx["vtm"]
        avps = []
        for half in range(2):          # half 0: h0,h1 ; half 1: h2,h3
            av = pm([128, 512])
            for j in range(2):
                h = 2 * half + j
                for cs in range(3):
                    nc.tensor.matmul(
                        av[64 * j:64 * j + 33, 0:T],
                        vtm[:, cs, h, :],
                        expt[:, cs, h, :],
                        start=(cs == 0), stop=(cs == 2))
            avps.append(av)
        rzf = [tiny.tile([1, 2, T], F32, tag=f"rz{half}", name=f"rz{half}")
               for half in range(2)]
        for half in range(2):
            for j in range(2):
                nc.vector.reciprocal(rzf[half][:, j, :],
                                     avps[half][64 * j + 32:64 * j + 33, 0:T])
        rzb = tiny.tile([1, 2, 2, T], BF16, tag="rzb")
        for half in range(2):
            nc.vector.tensor_copy(rzb[:, half], rzf[half])
        oat = sb.tile([128, T], BF16, tag="oat")
        for half in range(2):
            rzbc = pm([128, 512])
            for j in range(2):
                nc.tensor.matmul(rzbc[:, 0:T], hmask[:, j, :],
                                 rzb[:, half, j, :],
                                 start=(j == 0), stop=(j == 1))
            rzsb = sb.tile([128, T], F32, tag=f"rzsb{half}")
            nc.vector.tensor_copy(rzsb, rzbc[:, 0:T])
            for j in range(2):
                h = 2 * half + j
                nc.vector.tensor_tensor(oat[32 * h:32 * h + 32, :],
                                        avps[half][64 * j:64 * j + 32, 0:T],
                                        rzsb[64 * j:64 * j + 32, :],
                                        op=ALU.mult)
        cx["oat"] = oat

    def st_out(s, cx):
        rps = pm([128, 512])
        nc.tensor.matmul(rps[:, 0:T], owT, cx["oat"], start=True, stop=False)
        # residual: + h0 via identity matmul into the same psum
        nc.tensor.matmul(rps[:, 0:T], identb, cx["h0"], start=False, stop=True)
        rfm = sb.tile([128, T], BF16, tag="rfm")
        nc.scalar.activation(rfm, rps[:, 0:T], AF.Identity, bias=ob, scale=1.0)
        _, h1 = ln_norm(cx, rfm, "ln1", ln1g, ln1b, gelu=False)
        cx["h1"] = h1

    def st_ffn(s, cx):
        h1 = cx["h1"]
        g1 = sb.tile([128, 2, T], BF16, tag="g1")
        for i in range(2):
            f1ps = pm([128, 512])
            nc.tensor.matmul(f1ps[:, 0:T], f1T[i], h1, start=True, stop=True)
            nc.scalar.activation(g1[:, i], f1ps[:, 0:T], AF.Relu,
                                 bias=f1b[:, i:i + 1], scale=1.0)
        f2ps = pm([128, 512])
        for i in range(2):
            nc.tensor.matmul(f2ps[:, 0:T], f2T[i], g1[:, i],
                             start=(i == 0), stop=False)
        # residual: + h1 via identity matmul
        nc.tensor.matmul(f2ps[:, 0:T], identb, h1, start=False, stop=True)
        ffo = sb.tile([128, T], BF16, tag="ffo")
        nc.scalar.activation(ffo, f2ps[:, 0:T], AF.Identity, bias=f2b, scale=1.0)
        hat2, h2 = ln_norm(cx, ffo, "ln2", ln2g, ln2b, gelu=False)
        cx["hat2"] = hat2
        cx["h2"] = h2

    def st_pool(s, cx):
        plps = pm([1, 512])
        nc.tensor.matmul(plps[:, 0:T], poolw, cx["h2"], start=True, stop=True)
        pw_sb = tiny.tile([1, T], F32, tag="pw_sb")
        zp = tiny.tile([1, 1], F32, tag="zp")
        nc.scalar.activation(pw_sb, plps[:, 0:T], AF.Exp,
                             bias=poolb, scale=1.0, accum_out=zp)
        rzp = tiny.tile([1, 1], F32, tag="rzp")
        nc.vector.reciprocal(rzp, zp)
        wn = tiny.tile([1, T], BF16, tag="wn")
        nc.vector.tensor_scalar_mul(wn, pw_sb, rzp)
        wtp = pm([TCH, 3, 2], BF16)
        for c in range(3):
            nc.tensor.transpose(wtp[:, c, 0:1], wn[:, c * TCH:(c + 1) * TCH],
                                identb[0:1, 0:1])
        wcol = tiny.tile([TCH, 3, 1], BF16, tag="wcol")
        nc.vector.tensor_copy(wcol, wtp[:, :, 0:1])
        tps = pm([128, 1])
        for c in range(3):
            nc.tensor.matmul(tps, cx["hat2"][:, c, :], wcol[:, c, :],
                             start=(c == 0), stop=(c == 2))
        nc.vector.tensor_scalar(fus_t[:, s:s + 1], tps, ln2g, ln2b,
                                op0=ALU.mult, op1=ALU.add)

    def st_corr(s, cx):
        xmv = cx["xmv"]
        cvv = tiny.tile([90, 2], F32, tag="cvv")
        nc.gpsimd.tensor_single_scalar(cvv, xmv[:, :, 1],
                                       float(T) / (T - 1), op=ALU.mult)
        cinv = tiny.tile([90, 2], F32, tag="cinv")
        csc = tiny.tile([90, 2], F32, tag="cv_nsc")
        rsqrt_newton(cinv, cvv, csc)
        nc.gpsimd.tensor_single_scalar(cinv, cinv, 1e8, op=ALU.min)
        wcorr = sb.tile([90, 2, 12], BF16, tag="wcorr")
        for g in range(2):
            nc.vector.tensor_scalar_mul(wcorr[:, g], cmask[:, g],
                                        cinv[:, g:g + 1])
        swps = pm([12, 512])
        for g in range(2):
            nc.tensor.matmul(swps[:, 0:T], wcorr[:, g], cx["xfm"][:, g, :],
                             start=(g == 0), stop=(g == 1))
        swsb = tiny.tile([12, T], BF16, tag="swsb")
        rsum = tiny.tile([12, 1], F32, tag="rsum")
        nc.vector.tensor_scalar(swsb, swps[:, 0:T], 1.0, 0.0, op0=ALU.mult,
                                op1=ALU.add, accum_out=rsum)
        swtp = pm([TCH, 3, 12], BF16)
        for c in range(3):
            nc.tensor.transpose(swtp[:, c, :],
                                swsb[:, c * TCH:(c + 1) * TCH],
                                identb[0:12, 0:12])
        swtm = tiny.tile([TCH, 3, 12], BF16, tag="swtm")
        nc.vector.tensor_copy(swtm, swtp)
        rsb = tiny.tile([12, 1], BF16, tag="rsb")
        nc.vector.tensor_copy(rsb, rsum)
        rsT_ps = pm([1, 12], BF16)
        nc.tensor.transpose(rsT_ps, rsb, identb[0:12, 0:12])
        rsT = tiny.tile([1, 12], BF16, tag="rsT")
        nc.vector.tensor_copy(rsT, rsT_ps)
        rsTn = tiny.tile([1, 12], BF16, tag="rsTn")
        nc.vector.tensor_scalar_mul(rsTn, rsT, -1.0 / T)
        gps = pm([12, 12])
        for c in range(3):
            nc.tensor.matmul(gps, swtm[:, c, :], swtm[:, c, :],
                             start=(c == 0), stop=False)
        nc.tensor.matmul(gps, rsTn, rsT, start=False, stop=True)
        nc.vector.tensor_copy(bm_all[:, s, :], gps)

    STAGES = [st_ld, st_proj, st_conv, st_gn, st_merge, st_qkv,
              st_scores, st_av, st_out, st_ffn, st_pool, st_corr]

    ctxs = {}
    for pair in range(S // 2):
        sA, sB = 2 * pair, 2 * pair + 1
        ctxs[sA] = {}
        ctxs[sB] = {}
        for st in STAGES:
            st(sA, ctxs[sA])
            st(sB, ctxs[sB])
        del ctxs[sA], ctxs[sB]

    # ================= batched tail =================
    bm_dram = nc.dram_tensor("bm_scratch", [12, S, 12], F32).ap()
    dma.dma_start(out=bm_dram, in_=bm_all)
    fcv_f = W.tile([78, S], F32, tag="fcv_f")
    row_off = 0
    for i in range(12):
        n = 12 - i
        dma.dma_start(
            out=fcv_f[row_off:row_off + n, :],
            in_=bm_dram[i, :, i:12].transpose([1, 0]))
        row_off += n
    fcv = bcast("fcv", fcv_f)
    fcps = pm([64, S])
    nc.tensor.matmul(fcps, fcwk, fcv, start=True, stop=True)
    nc.scalar.activation(fus_f, fcps, AF.Gelu, bias=fcb, scale=1.0)

    fu_ps = pm([128, S])
    nc.tensor.matmul(fu_ps, fu1T[:, 0, :], fus_t, start=True, stop=False)
    nc.tensor.matmul(fu_ps, fu1T[0:64, 1, :], fus_f, start=False, stop=True)
    zfm = W.tile([128, S], BF16, tag="zfm")
    nc.vector.tensor_scalar(zfm, fu_ps, 1.0, fu1b, op0=ALU.mult, op1=ALU.add)
    ztp = pm([S, 128], BF16)
    nc.tensor.transpose(ztp, zfm, identb)
    ztm = W.tile([S, 128], F32, tag="ztm")
    nc.vector.tensor_copy(ztm, ztp)
    zst = W.tile([S, 6], F32, tag="zst")
    nc.vector.bn_stats(zst, ztm)
    zmv = W.tile([S, 2], F32, tag="zmv")
    nc.vector.bn_aggr(zmv, zst)
    zvpe = W.tile([S, 1], F32, tag="zvpe")
    nc.vector.tensor_scalar(zvpe, zmv[:, 1:2], EPS, None, op0=ALU.add)
    zrs = W.tile([S, 1], F32, tag="zrs")
    zsc = W.tile([S, 1], F32, tag="zsc")
    rsqrt_newton(zrs, zvpe, zsc)
    zhat = W.tile([S, 128], BF16, tag="zhat")
    nc.vector.tensor_scalar(zhat, ztm, zmv[:, 0:1], zrs,
                            op0=ALU.subtract, op1=ALU.mult)
    zhtp = pm([128, S], BF16)
    nc.tensor.transpose(zhtp, zhat, identb[0:S, 0:S])
    zg = W.tile([128, S], BF16, tag="zg")
    nc.scalar.activation(zg, zhtp, AF.Gelu, bias=flb, scale=flg)
    out_ps = pm([64, S])
    nc.tensor.matmul(out_ps, fu2T, zg, start=True, stop=True)
    out_sb = W.tile([64, S], BF16, tag="out_sb")
    nc.scalar.activation(out_sb, out_ps, AF.Identity, bias=fu2b, scale=1.0)
    outT_ps = pm([S, 64], BF16)
    nc.tensor.transpose(outT_ps, out_sb, identb[0:64, 0:64])
    outT = W.tile([S, 64], F32, tag="outT")
    nc.vector.tensor_copy(outT, outT_ps)
    dma.dma_start(out=out_dram, in_=outT)

    for p in reversed(pools):
        p.__exit__(None, None, None)


_PROGRAM = None


def _get_program():
    global _PROGRAM
    if _PROGRAM is None:
        _PROGRAM = build_program()
    return _PROGRAM


def kernel(**inputs):
    from concourse.bass_utils import run_bass_kernel_spmd

    nc, _ = _get_program()
    in_maps = []
    for c in range(NCORES):
        m = {}
        for name, _shape in INPUT_SPECS:
            if name == "x":
                m["x"] = np.ascontiguousarray(
                    np.asarray(inputs["x"][c * S:(c + 1) * S], dtype=np.float32))
            else:
                m[name] = np.ascontiguousarray(
                    np.asarray(inputs[name], dtype=np.float32))
        in_maps.append(m)
    res = run_bass_kernel_spmd(nc, in_maps, list(range(NCORES)))
    global LAST_RESULTS
    LAST_RESULTS = res
    out = np.concatenate([res.results[c]["out"] for c in range(NCORES)], axis=0)
    return out.astype(np.float32)


LAST_RESULTS = None


# revision 20
# speedup vs baseline: 1.6055x; 1.3503x over previous
"""Trainium2 Bass kernel for nn_NetworkAwareClassicalExpert (B=256,T=363,C=180).

Data-parallel over 8 NeuronCores: 32 samples/core. Per-core program processes
samples in PAIRS with stage-level interleaving so engines overlap across the
two in-flight samples.

vs the original single-sample version:
  - all large matmuls and transposes in bf16 (fp32 matmuls/transposes lower
    to 2 HW matmuls each; bf16 streams 1 col/cycle at any size). Stats, LN
    scalar math, and the tiny GroupNorm aggregation matmuls stay fp32.
  - attention AV computed s-major (P as moving rhs): 12 matmuls with the
    softmax normalizer Z from a ones-column in V; per-head reciprocal +
    mask-matmul broadcast replaces token-major normalization + transposes
  - conv bias folded into the GroupNorm affine (no per-sample bias matmuls)
  - residual adds folded into PSUM via identity matmuls
  - per-sample rsqrt via bit-hack + Newton iterations (DVE init, GpSimd
    iterations) so the Scalar engine only loads the Gelu and Exp activation
    tables (2 loads per sample pair, vs ~8 table loads per sample)
"""

import sys
import os

sys.path.insert(0, "/opt/trn_rl_repo")

import numpy as np

import concourse.bass as bass
import concourse.mybir as mybir
import concourse.tile as tile
import bass_rust
from concourse.vector_clock import ScopedClock
from concourse.masks import make_identity

F32 = mybir.dt.float32
BF16 = mybir.dt.bfloat16
I32 = mybir.dt.int32
AF = mybir.ActivationFunctionType
ALU = mybir.AluOpType

B, T, C = 256, 363, 180
CD = 96
DM = 128
DILS = (1, 4, 16)
NCORES = 8
S = int(os.environ.get("KB_NSAMP", str(B // NCORES)))
TCH = 121                # t-chunk (3 chunks of 121)
PAD = 48
EPS = 1e-5
ISQ = float(1.0 / np.sqrt(32.0))
MAGIC = 0x5F3759DF

DEBUG = bool(int(os.environ.get("KBDBG", "0")))
DBG_SAMPLES = int(os.environ.get("KBDBG_S", "2"))


def _patch_tile_drain():
    """This walrus rejects >1 sem wait on the final Tile drain: split them."""

    def _drain_and_barrier(self, tick_clock, wait_clock):
        drain_inst = self.nc.sync.drain()
        wait_clock.add_sem_waits(
            drain_inst.ins, ScopedClock({None: tick_clock.global_clock})
        )
        si = drain_inst.ins.sync_info
        if si is not None and si.on_wait is not None and len(si.on_wait) > 1:
            waits = list(si.on_wait)
            ups = list(si.on_update) if si.on_update else []
            drain_inst.ins.sync_info = bass_rust.SyncInfo(
                on_wait=waits[:1], on_update=ups
            )
            for w in waits[1:]:
                nop = self.nc.sync.nop()
                nop.ins.sync_info = bass_rust.SyncInfo(on_wait=[w], on_update=[])
        self.nc.all_engine_barrier()
        popped = self.nc._tile_sem_poison_stack.pop()
        assert popped is self._sem_poison
        if not int(os.environ.get("KB_NOSEMCLEAR", "0")):
            self.nc.clear_and_free_semaphores(list(self.sems.allocated().values()))
        self.nc.all_engine_barrier()

    tile.TileContext._drain_and_barrier = _drain_and_barrier


_patch_tile_drain()


def _split_sync_waits(nc, max_waits=1):
    """walrus rejects instructions with >1 sem wait; hoist excess onto
    same-engine NOPs inserted immediately before (created via the real
    bass emitters so they carry all rust-side init)."""
    for f in nc.m.functions:
        for bb in f.blocks:
            insts = list(bb.instructions)
            out = []
            changed = False
            for inst in insts:
                si = getattr(inst, "sync_info", None)
                if si is not None and si.on_wait and len(si.on_wait) > max_waits:
                    waits = list(si.on_wait)
                    ups = list(si.on_update) if si.on_update else []
                    extra = waits[max_waits:]
                    for i in range(0, len(extra), max_waits):
                        nop = nc.engines[inst.engine].nop(nofuse=True)
                        cur = nn_cur_bb(nc)
                        lst = list(cur.instructions)
                        assert lst and lst[-1].name == nop.ins.name
                        cur.instructions = lst[:-1]
                        nop.ins.sync_info = bass_rust.SyncInfo(
                            on_wait=extra[i:i + max_waits], on_update=[])
                        out.append(nop.ins)
                    inst.sync_info = bass_rust.SyncInfo(
                        on_wait=waits[:max_waits], on_update=ups)
                    changed = True
                out.append(inst)
            if changed:
                bb.instructions = out


def nn_cur_bb(nc):
    bbw = nc.cur_bb
    return bbw.bb if hasattr(bbw, "bb") else bbw


INPUT_SPECS = [
    ("x", (S, T, C)),
    ("w_proj", (12, 15, 8)), ("b_proj", (12, 8)),
    ("dw_w", (3, 96, 7)), ("dw_b", (3, 96)),
    ("pw_w", (3, 96, 96)), ("pw_b", (3, 96)),
    ("gn_g", (3, 96)), ("gn_b", (3, 96)),
    ("merge_w", (288, 128)), ("merge_b", (128,)),
    ("merge_ln_g", (128,)), ("merge_ln_b", (128,)),
    ("qkv_w", (384, 128)), ("qkv_b", (384,)),
    ("out_w", (128, 128)), ("out_b", (128,)),
    ("ln1_g", (128,)), ("ln1_b", (128,)),
    ("ff1_w", (256, 128)), ("ff1_b", (256,)),
    ("ff2_w", (128, 256)), ("ff2_b", (128,)),
    ("ln2_g", (128,)), ("ln2_b", (128,)),
    ("pool_w", (128, 1)), ("pool_b", (1,)),
    ("fc_w", (78, 64)), ("fc_b", (64,)),
    ("fus1_w", (192, 128)), ("fus1_b", (128,)),
    ("fus_ln_g", (128,)), ("fus_ln_b", (128,)),
    ("fus2_w", (128, 64)), ("fus2_b", (64,)),
]


def build_program():
    nc = bass.Bass("TRN2", target_bir_lowering=False, debug=False,
                   num_devices=NCORES)
    D = {}
    for name, shape in INPUT_SPECS:
        D[name] = nc.dram_tensor(name, list(shape), F32, kind="ExternalInput").ap()
    out_dram = nc.dram_tensor("out", [S, 64], F32, kind="ExternalOutput").ap()

    with tile.TileContext(nc) as tc:
        _build(nc, tc, D, out_dram)
    if not int(os.environ.get("KB_NOSPLIT", "0")):
        _split_sync_waits(nc)
    return nc, {}


def _build(nc, tc, D, out_dram):
    pools = []

    def mkpool(name, bufs, space="SBUF"):
        p = tc.tile_pool(name=name, bufs=bufs, space=space)
        pools.append(p)
        return p.__enter__()

    W = mkpool("weights", 1)        # persistent tiles, one tag each
    early = mkpool("early", 4)      # front-of-pipe tiles (next-pair prefetch)
    sb = mkpool("sb", 2)            # per-sample activations
    tiny = mkpool("tiny", 2)        # small per-sample stats
    Psc = mkpool("psc", 1, "PSUM")  # scores [121,2,512] -> 2 banks
    Pcv = mkpool("pcv", 2, "PSUM")  # conv/proj [96,512]  -> 2 banks
    Pm = mkpool("pm", 4, "PSUM")    # everything else [128,512] -> 4 banks

    dma = nc.sync

    _pm_n = [0]

    def pm(shape, dtype=F32):
        _pm_n[0] += 1
        return Pm.tile(list(shape), dtype, tag="pm", name=f"pm{_pm_n[0]}")

    def rsqrt_newton(y, v, sc, n_iter=1):
        """y <- rsqrt(v) elementwise. y, v, sc float32 SBUF tiles, same shape.
        Bit-hack init + Newton iterations, all on DVE (short serial chain)."""
        yb = y.bitcast(I32)
        vb = v.bitcast(I32)
        nc.vector.tensor_scalar(yb, vb, 1, -1, op0=ALU.logical_shift_right,
                                op1=ALU.bitwise_xor)
        nc.vector.tensor_scalar(yb, yb, MAGIC + 1, None, op0=ALU.add)
        for _ in range(n_iter):
            nc.vector.tensor_tensor(sc, y, y, op=ALU.mult)
            nc.vector.scalar_tensor_tensor(sc, v, -0.5, sc,
                                           op0=ALU.mult, op1=ALU.mult)
            nc.vector.scalar_tensor_tensor(y, sc, 1.5, y,
                                           op0=ALU.add, op1=ALU.mult)

    def bcast(name, src):
        """Cast an f32 W-tile to a bf16 W-tile."""
        t = W.tile(list(src.shape), BF16, tag=f"{name}_b", name=f"{name}_b")
        nc.vector.tensor_copy(t, src)
        return t

    # ================= weight preload =================
    ident = W.tile([128, 128], F32, tag="ident")
    make_identity(nc, ident)
    identb = bcast("ident", ident)

    wproj_f = W.tile([90, 2, 96], F32, tag="wproj_f")
    nc.vector.memset(wproj_f, 0.0)
    for n in range(12):
        g, j = divmod(n, 6)
        dma.dma_start(out=wproj_f[j * 15:(j + 1) * 15, g, n * 8:(n + 1) * 8],
                      in_=D["w_proj"][n])
    wproj = bcast("wproj", wproj_f)
    bproj = W.tile([96, 1], F32, tag="bproj")
    dma.dma_start(out=bproj, in_=D["b_proj"].rearrange("a b -> (a b)").unsqueeze(1))

    pwT, dwk = [], []
    for k in range(3):
        t_ = W.tile([96, 96], F32, tag=f"pwT{k}")
        dma.dma_start(out=t_, in_=D["pw_w"][k].transpose([1, 0]))
        pwT.append(t_)
        t2 = W.tile([96, 7], F32, tag=f"dw{k}")
        dma.dma_start(out=t2, in_=D["dw_w"][k])
        dwk.append(t2)
    wconv = []
    for k in range(3):
        t_ = W.tile([96, 7, 96], BF16, tag=f"wconv{k}")
        for j in range(7):
            nc.vector.tensor_scalar_mul(t_[:, j, :], pwT[k], dwk[k][:, j:j + 1])
        wconv.append(t_)
    dwb = W.tile([96, 3], F32, tag="dwb")
    dma.dma_start(out=dwb, in_=D["dw_b"].transpose([1, 0]))
    pwb = W.tile([96, 3], F32, tag="pwb")
    dma.dma_start(out=pwb, in_=D["pw_b"].transpose([1, 0]))
    cb_ps = pm([96, 3])
    for k in range(3):
        nc.tensor.matmul(cb_ps[:, k:k + 1], pwT[k], dwb[:, k:k + 1],
                         start=True, stop=True, skip_group_check=True)
    cb = W.tile([96, 3], F32, tag="cb")
    nc.vector.tensor_add(cb, cb_ps, pwb)

    gng = W.tile([96, 3], F32, tag="gng")
    dma.dma_start(out=gng, in_=D["gn_g"].transpose([1, 0]))
    gnb = W.tile([96, 3], F32, tag="gnb")
    dma.dma_start(out=gnb, in_=D["gn_b"].transpose([1, 0]))

    # wgrp[c, g] = 1/12 iff 0 <= c - 12g <= 11 ; wbc[g, c] = 1 iff same
    wgrp = W.tile([96, 8], F32, tag="wgrp")
    nc.vector.memset(wgrp, 1.0 / 12.0)
    nc.gpsimd.affine_select(out=wgrp, in_=wgrp, compare_op=ALU.is_ge,
                            fill=0.0, base=0, pattern=[[-12, 8]],
                            channel_multiplier=1)
    nc.gpsimd.affine_select(out=wgrp, in_=wgrp, compare_op=ALU.is_ge,
                            fill=0.0, base=11, pattern=[[12, 8]],
                            channel_multiplier=-1)
    wbc = W.tile([8, 96], F32, tag="wbc")
    nc.vector.memset(wbc, 1.0)
    nc.gpsimd.affine_select(out=wbc, in_=wbc, compare_op=ALU.is_ge,
                            fill=0.0, base=0, pattern=[[1, 96]],
                            channel_multiplier=-12)
    nc.gpsimd.affine_select(out=wbc, in_=wbc, compare_op=ALU.is_ge,
                            fill=0.0, base=11, pattern=[[-1, 96]],
                            channel_multiplier=12)

    mw_f = W.tile([96, 3, 128], F32, tag="mw_f")
    for g in range(3):
        dma.dma_start(out=mw_f[:, g, :], in_=D["merge_w"][g * 96:(g + 1) * 96, :])
    mw = bcast("mw", mw_f)
    mb = W.tile([128, 1], F32, tag="mb")
    dma.dma_start(out=mb, in_=D["merge_b"].unsqueeze(1))
    mlng = W.tile([128, 1], F32, tag="mlng")
    dma.dma_start(out=mlng, in_=D["merge_ln_g"].unsqueeze(1))
    mlnb = W.tile([128, 1], F32, tag="mlnb")
    dma.dma_start(out=mlnb, in_=D["merge_ln_b"].unsqueeze(1))

    qkvT_f = W.tile([128, 3, 128], F32, tag="qkvT_f")
    for i in range(3):
        dma.dma_start(out=qkvT_f[:, i, :],
                      in_=D["qkv_w"][i * 128:(i + 1) * 128, :].transpose([1, 0]))
    qkvT = bcast("qkvT", qkvT_f)
    qb3 = W.tile([128, 3], F32, tag="qb3")
    dma.dma_start(out=qb3, in_=D["qkv_b"].rearrange("(a b) -> b a", a=3))
    qb_s = W.tile([128, 1], F32, tag="qb_s")
    nc.vector.tensor_scalar_mul(qb_s, qb3[:, 0:1], ISQ)

    owT_f = W.tile([128, 128], F32, tag="owT_f")
    dma.dma_start(out=owT_f, in_=D["out_w"].transpose([1, 0]))
    owT = bcast("owT", owT_f)
    ob = W.tile([128, 1], F32, tag="ob")
    dma.dma_start(out=ob, in_=D["out_b"].unsqueeze(1))

    ln1g = W.tile([128, 1], F32, tag="ln1g")
    dma.dma_start(out=ln1g, in_=D["ln1_g"].unsqueeze(1))
    ln1b = W.tile([128, 1], F32, tag="ln1b")
    dma.dma_start(out=ln1b, in_=D["ln1_b"].unsqueeze(1))
    ln2g = W.tile([128, 1], F32, tag="ln2g")
    dma.dma_start(out=ln2g, in_=D["ln2_g"].unsqueeze(1))
    ln2b = W.tile([128, 1], F32, tag="ln2b")
    dma.dma_start(out=ln2b, in_=D["ln2_b"].unsqueeze(1))

    f12_f = W.tile([128, 4, 128], F32, tag="f12_f")
    for i in range(2):
        dma.dma_start(out=f12_f[:, i, :],
                      in_=D["ff1_w"][i * 128:(i + 1) * 128, :].transpose([1, 0]))
        dma.dma_start(out=f12_f[:, 2 + i, :],
                      in_=D["ff2_w"][:, i * 128:(i + 1) * 128].transpose([1, 0]))
    f12 = bcast("f12", f12_f)
    f1T = [f12[:, 0, :], f12[:, 1, :]]
    f2T = [f12[:, 2, :], f12[:, 3, :]]
    f1b = W.tile([128, 2], F32, tag="f1b")
    dma.dma_start(out=f1b, in_=D["ff1_b"].rearrange("(a b) -> b a", a=2))
    f2b = W.tile([128, 1], F32, tag="f2b")
    dma.dma_start(out=f2b, in_=D["ff2_b"].unsqueeze(1))

    poolw_f = W.tile([128, 1], F32, tag="poolw_f")
    dma.dma_start(out=poolw_f, in_=D["pool_w"])
    poolw = bcast("poolw", poolw_f)
    poolb = W.tile([1, 1], F32, tag="poolb")
    dma.dma_start(out=poolb, in_=D["pool_b"].unsqueeze(1))

    # cmask[p, g, n] = 1 iff 0 <= p - 15*(n - 6g) <= 14
    cmask = W.tile([90, 2, 12], F32, tag="cmask")
    nc.vector.memset(cmask, 1.0)
    nc.gpsimd.affine_select(out=cmask, in_=cmask, compare_op=ALU.is_ge,
                            fill=0.0, base=0, pattern=[[90, 2], [-15, 12]],
                            channel_multiplier=1)
    nc.gpsimd.affine_select(out=cmask, in_=cmask, compare_op=ALU.is_ge,
                            fill=0.0, base=14, pattern=[[-90, 2], [15, 12]],
                            channel_multiplier=-1)
    kcorr = float(1.0 / (15 * 15 * (T - 1)))
    fcw = W.tile([78, 64], F32, tag="fcw")
    dma.dma_start(out=fcw, in_=D["fc_w"])
    fcwk = W.tile([78, 64], BF16, tag="fcwk")
    nc.vector.tensor_scalar_mul(fcwk, fcw, kcorr)
    fcb = W.tile([64, 1], F32, tag="fcb")
    dma.dma_start(out=fcb, in_=D["fc_b"].unsqueeze(1))

    fu1T_f = W.tile([128, 2, 128], F32, tag="fu1T_f")
    nc.vector.memset(fu1T_f[:, 1, :], 0.0)
    dma.dma_start(out=fu1T_f[:, 0, :], in_=D["fus1_w"][0:128, :])
    dma.dma_start(out=fu1T_f[0:64, 1, :], in_=D["fus1_w"][128:192, :])
    fu1T = bcast("fu1T", fu1T_f)
    fu1b = W.tile([128, 1], F32, tag="fu1b")
    dma.dma_start(out=fu1b, in_=D["fus1_b"].unsqueeze(1))
    flg = W.tile([128, 1], F32, tag="flg")
    dma.dma_start(out=flg, in_=D["fus_ln_g"].unsqueeze(1))
    flb = W.tile([128, 1], F32, tag="flb")
    dma.dma_start(out=flb, in_=D["fus_ln_b"].unsqueeze(1))
    fu2T_f = W.tile([128, 64], F32, tag="fu2T_f")
    dma.dma_start(out=fu2T_f, in_=D["fus2_w"])
    fu2T = bcast("fu2T", fu2T_f)
    fu2b = W.tile([64, 1], F32, tag="fu2b")
    dma.dma_start(out=fu2b, in_=D["fus2_b"].unsqueeze(1))

    bm_all = W.tile([12, S, 12], F32, tag="bm_all")
    fus_t = W.tile([128, S], BF16, tag="fus_t")
    fus_f = W.tile([64, S], BF16, tag="fus_f")

    # ================= per-sample stages =================

    def ln_norm(cx, src_fm, tag, gout, bout, gelu):
        """LN over features of src_fm [128, T] (bf16). -> (hat_tm, out_fm)."""
        tp = pm([TCH, 3, 128], BF16)
        for c in range(3):
            nc.tensor.transpose(tp[:, c, :],
                                src_fm[:, c * TCH:(c + 1) * TCH], identb)
        utm = sb.tile([TCH, 3, 128], F32, tag=f"{tag}_tm")
        nc.vector.tensor_copy(utm, tp)
        stats = tiny.tile([TCH, 3, 6], F32, tag=f"{tag}_st")
        for c in range(3):
            nc.vector.bn_stats(stats[:, c], utm[:, c])
        mv = tiny.tile([TCH, 3, 2], F32, tag=f"{tag}_mv")
        for c in range(3):
            nc.vector.bn_aggr(mv[:, c], stats[:, c])
        vpe = tiny.tile([TCH, 3], F32, tag=f"{tag}_vpe")
        nc.vector.tensor_scalar(vpe, mv[:, :, 1], EPS, None, op0=ALU.add)
        rs = tiny.tile([TCH, 3], F32, tag=f"{tag}_rs")
        nsc = tiny.tile([TCH, 3], F32, tag=f"{tag}_nsc")
        rsqrt_newton(rs, vpe, nsc)
        hat = sb.tile([TCH, 3, 128], BF16, tag=f"{tag}_hat")
        for c in range(3):
            nc.vector.tensor_scalar(hat[:, c], utm[:, c],
                                    mv[:, c, 0:1], rs[:, c:c + 1],
                                    op0=ALU.subtract, op1=ALU.mult)
        tp2 = pm([128, 3, 128], BF16)
        for c in range(3):
            nc.tensor.transpose(tp2[:, c, 0:TCH], hat[:, c, :],
                                identb[0:TCH, 0:TCH])
        ofm = sb.tile([128, T], BF16, tag=f"{tag}_ofm")
        nc.scalar.activation(
            ofm.rearrange("a (c t) -> a c t", c=3), tp2[:, :, 0:TCH],
            AF.Gelu if gelu else AF.Identity, bias=bout, scale=gout)
        return hat, ofm

    def st_ld(s, cx):
        xtm = early.tile([TCH, 3, C], F32, tag="xtm")
        dma.dma_start(out=xtm, in_=D["x"][s].rearrange("(c p) f -> p c f", p=TCH))
        xtb = early.tile([TCH, 3, C], BF16, tag="xtb")
        nc.vector.tensor_copy(xtb, xtm)
        xfm = early.tile([90, 2, T], BF16, tag="xfm")
        for g in range(2):
            xps = pm([90, 3, 128], BF16)
            for c in range(3):
                nc.tensor.transpose(xps[:, c, 0:TCH],
                                    xtb[:, c, g * 90:(g + 1) * 90],
                                    identb[0:TCH, 0:TCH])
            nc.vector.tensor_copy(
                xfm[:, g, :].rearrange("a (c t) -> a c t", c=3),
                xps[:, :, 0:TCH])
        xst = tiny.tile([90, 2, 3, 6], F32, tag="xst")
        for g in range(2):
            for c in range(3):
                nc.vector.bn_stats(
                    xst[:, g, c], xfm[:, g, c * TCH:(c + 1) * TCH])
        xmv = tiny.tile([90, 2, 2], F32, tag="xmv")
        for g in range(2):
            nc.vector.bn_aggr(xmv[:, g], xst[:, g])
        cx["xfm"] = xfm
        cx["xmv"] = xmv

    def st_proj(s, cx):
        hps = Pcv.tile([96, 512], F32, tag="cv", name="hps")
        for g in range(2):
            nc.tensor.matmul(hps[:, 0:T], wproj[:, g, :], cx["xfm"][:, g, :],
                             start=(g == 0), stop=(g == 1))
        hpad = early.tile([96, PAD + T + PAD + 5], BF16, tag="hpad")
        nc.gpsimd.memset(hpad[:, 0:PAD], 0.0)
        nc.gpsimd.memset(hpad[:, PAD + T:], 0.0)
        nc.scalar.activation(hpad[:, PAD:PAD + T], hps[:, 0:T], AF.Gelu,
                             bias=bproj, scale=1.0)
        cx["hpad"] = hpad

    def st_conv(s, cx):
        ysb = sb.tile([96, 3, T], F32, tag="ysb")
        hpad = cx["hpad"]
        for k in range(3):
            d = DILS[k]
            yps = Pcv.tile([96, 512], F32, tag="cv", name=f"yps{k}")
            for j in range(7):
                off = PAD + (j - 3) * d
                nc.tensor.matmul(yps[:, 0:364], wconv[k][:, j, :],
                                 hpad[:, off:off + 364],
                                 start=(j == 0), stop=(j == 6))
            nc.vector.tensor_copy(ysb[:, k], yps[:, 0:T])
        yst = tiny.tile([96, 3, 6], F32, tag="yst")
        for k in range(3):
            nc.vector.bn_stats(yst[:, k], ysb[:, k])
        ymv = tiny.tile([96, 3, 2], F32, tag="ymv")
        for k in range(3):
            nc.vector.bn_aggr(ymv[:, k], yst[:, k])
        cx["ysb"] = ysb
        cx["ymv"] = ymv

    def st_gn(s, cx):
        ymv = cx["ymv"]
        st6 = tiny.tile([96, 6], F32, tag="st6")
        # true per-channel mean m' = m_nobias + cb ; E[y^2] = m'^2 + var
        nc.gpsimd.tensor_tensor(st6[:, 0:3], ymv[:, :, 0], cb, op=ALU.add)
        nc.gpsimd.tensor_tensor(st6[:, 3:6], st6[:, 0:3], st6[:, 0:3],
                                op=ALU.mult)
        nc.gpsimd.tensor_tensor(st6[:, 3:6], st6[:, 3:6], ymv[:, :, 1],
                                op=ALU.add)
        gst_ps = pm([8, 6])
        nc.tensor.matmul(gst_ps, wgrp, st6, start=True, stop=True)
        gst = tiny.tile([8, 6], F32, tag="gst")
        nc.vector.tensor_copy(gst, gst_ps)
        gvar = tiny.tile([8, 3], F32, tag="gvar")
        # var_g + eps = E[y^2]_g - m_g^2 + eps
        nc.vector.tensor_tensor(gvar, gst[:, 0:3], gst[:, 0:3], op=ALU.mult)
        nc.vector.tensor_tensor(gvar, gst[:, 3:6], gvar, op=ALU.subtract)
        nc.vector.tensor_scalar(gvar, gvar, EPS, None, op0=ALU.add)
        gsc = tiny.tile([8, 3], F32, tag="gv_nsc")
        rsqrt_newton(gst[:, 3:6], gvar, gsc)
        bc_ps = pm([96, 6])
        nc.tensor.matmul(bc_ps, wbc, gst, start=True, stop=True)
        scl = tiny.tile([96, 3], F32, tag="scl")
        nc.vector.tensor_tensor(scl, gng, bc_ps[:, 3:6], op=ALU.mult)
        bia = tiny.tile([96, 3], F32, tag="bia")
        # bia = gnb + (cb - gm) * scl
        nc.vector.tensor_tensor(bia, cb, bc_ps[:, 0:3], op=ALU.subtract)
        nc.gpsimd.tensor_tensor(bia, bia, scl, op=ALU.mult)
        nc.gpsimd.tensor_tensor(bia, bia, gnb, op=ALU.add)
        cat = sb.tile([96, 3, T], BF16, tag="cat")
        for k in range(3):
            nc.scalar.activation(cat[:, k], cx["ysb"][:, k], AF.Gelu,
                                 bias=bia[:, k:k + 1], scale=scl[:, k:k + 1])
        cx["cat"] = cat

    def st_merge(s, cx):
        ups = pm([128, 512])
        for g in range(3):
            nc.tensor.matmul(ups[:, 0:T], mw[:, g, :], cx["cat"][:, g],
                             start=(g == 0), stop=(g == 2))
        ufm = sb.tile([128, T], BF16, tag="ufm")
        nc.scalar.activation(ufm, ups[:, 0:T], AF.Identity, bias=mb, scale=1.0)
        _, h0 = ln_norm(cx, ufm, "mln", mlng, mlnb, gelu=True)
        cx["h0"] = h0

    def st_qkv(s, cx):
        h0 = cx["h0"]
        qps, kps, vps = pm([128, 512]), pm([128, 512]), pm([128, 512])
        nc.tensor.matmul(qps[:, 0:T], qkvT[:, 0, :], h0, start=True, stop=True)
        nc.tensor.matmul(kps[:, 0:T], qkvT[:, 1, :], h0, start=True, stop=True)
        nc.tensor.matmul(vps[:, 0:T], qkvT[:, 2, :], h0, start=True, stop=True)
        qfm = sb.tile([64, 2, T], BF16, tag="qfm")
        kfm = sb.tile([64, 2, T], BF16, tag="kfm")
        for i in range(2):
            nc.scalar.activation(qfm[:, i, :], qps[i * 64:(i + 1) * 64, 0:T],
                                 AF.Identity, bias=qb_s[i * 64:(i + 1) * 64, :],
                                 scale=ISQ)
            nc.vector.tensor_scalar(kfm[:, i, :], kps[i * 64:(i + 1) * 64, 0:T],
                                    1.0, qb3[i * 64:(i + 1) * 64, 1:2],
                                    op0=ALU.mult, op1=ALU.add)
        vfm = sb.tile([128, T], BF16, tag="vfm")
        nc.vector.tensor_scalar(vfm, vps[:, 0:T], 1.0, qb3[:, 2:3],
                                op0=ALU.mult, op1=ALU.add)
        vtp = pm([TCH, 3, 128], BF16)
        for c in range(3):
            nc.tensor.transpose(vtp[:, c, :],
                                vfm[:, c * TCH:(c + 1) * TCH], identb)
        vtm = sb.tile([TCH, 3, 4, 33], BF16, tag="vtm")
        nc.vector.tensor_copy(vtm[:, :, :, 0:32],
                              vtp.rearrange("p c (h d) -> p c h d", h=4))
        nc.gpsimd.memset(vtm[:, :, :, 32:33], 1.0)
        cx["qfm"] = qfm
        cx["kfm"] = kfm
        cx["vtm"] = vtm

    def st_scores(s, cx):
        qfm, kfm = cx["qfm"], cx["kfm"]
        expt = sb.tile([TCH, 3, 4, T], BF16, tag="expt")
        for cs in range(3):
            for hp in range(2):
                scps = Psc.tile([TCH, 2, 512], F32, tag="sc", name="scps")
                for hh in range(2):
                    h = 2 * hp + hh
                    i, b = divmod(h, 2)
                    nc.tensor.matmul(
                        scps[:, hh, 0:T],
                        kfm[b * 32:(b + 1) * 32, i, cs * TCH:(cs + 1) * TCH],
                        qfm[b * 32:(b + 1) * 32, i, :],
                        start=True, stop=True)
                nc.scalar.activation(expt[:, cs, 2 * hp:2 * hp + 2, :],
                                     scps[:, :, 0:T], AF.Exp)
        cx["expt"] = expt

    def st_av(s, cx):
        expt, vtm = cx["expt"], cx["vtm"]
        on_tm = sb.tile([TCH, 3, 128], BF16, tag="on_tm")
        rz = tiny.tile([TCH, 3, 4], F32, tag="rz")
        for ct in range(3):
            ops_ = pm([TCH, 4, 33])
            for h in range(4):
                for cs in range(3):
                    nc.tensor.matmul(ops_[:, h, :],
                                     expt[:, cs, h, ct * TCH:(ct + 1) * TCH],
                                     vtm[:, cs, h, :],
                                     start=(cs == 0), stop=(cs == 2))
            otm = tiny.tile([TCH, 4, 33], F32, tag="otm")
            nc.vector.tensor_copy(otm, ops_)
            nc.vector.reciprocal(rz[:, ct], otm[:, :, 32])
            for h in range(4):
                nc.vector.tensor_scalar_mul(on_tm[:, ct, h * 32:(h + 1) * 32],
                                            otm[:, h, 0:32],
                                            rz[:, ct, h:h + 1])
        otp = pm([128, 3, 128], BF16)
        for ct in range(3):
            nc.tensor.transpose(otp[:, ct, 0:TCH], on_tm[:, ct, :],
                                identb[0:TCH, 0:TCH])
        oat = sb.tile([128, T], BF16, tag="oat")
        nc.vector.tensor_copy(oat.rearrange("a (c t) -> a c t", c=3),
                              otp[:, :, 0:TCH])
        cx["oat"] = oat

    def st_out(s, cx):
        rps = pm([128, 512])
        nc.tensor.matmul(rps[:, 0:T], owT, cx["oat"], start=True, stop=False)
        # residual: + h0 via identity matmul into the same psum
        nc.tensor.matmul(rps[:, 0:T], identb, cx["h0"], start=False, stop=True)
        rfm = sb.tile([128, T], BF16, tag="rfm")
        nc.scalar.activation(rfm, rps[:, 0:T], AF.Identity, bias=ob, scale=1.0)
        _, h1 = ln_norm(cx, rfm, "ln1", ln1g, ln1b, gelu=False)
        cx["h1"] = h1

    def st_ffn(s, cx):
        h1 = cx["h1"]
        g1 = sb.tile([128, 2, T], BF16, tag="g1")
        for i in range(2):
            f1ps = pm([128, 512])
            nc.tensor.matmul(f1ps[:, 0:T], f1T[i], h1, start=True, stop=True)
            nc.scalar.activation(g1[:, i], f1ps[:, 0:T], AF.Relu,
                                 bias=f1b[:, i:i + 1], scale=1.0)
        f2ps = pm([128, 512])
        for i in range(2):
            nc.tensor.matmul(f2ps[:, 0:T], f2T[i], g1[:, i],
                             start=(i == 0), stop=False)
        # residual: + h1 via identity matmul
        nc.tensor.matmul(f2ps[:, 0:T], identb, h1, start=False, stop=True)
        ffo = sb.tile([128, T], BF16, tag="ffo")
        nc.scalar.activation(ffo, f2ps[:, 0:T], AF.Identity, bias=f2b, scale=1.0)
        hat2, h2 = ln_norm(cx, ffo, "ln2", ln2g, ln2b, gelu=False)
        cx["hat2"] = hat2
        cx["h2"] = h2

    def st_pool(s, cx):
        plps = pm([1, 512])
        nc.tensor.matmul(plps[:, 0:T], poolw, cx["h2"], start=True, stop=True)
        pw_sb = tiny.tile([1, T], F32, tag="pw_sb")
        zp = tiny.tile([1, 1], F32, tag="zp")
        nc.scalar.activation(pw_sb, plps[:, 0:T], AF.Exp,
                             bias=poolb, scale=1.0, accum_out=zp)
        rzp = tiny.tile([1, 1], F32, tag="rzp")
        nc.vector.reciprocal(rzp, zp)
        wn = tiny.tile([1, T], BF16, tag="wn")
        nc.vector.tensor_scalar_mul(wn, pw_sb, rzp)
        wtp = pm([TCH, 3, 2], BF16)
        for c in range(3):
            nc.tensor.transpose(wtp[:, c, 0:1], wn[:, c * TCH:(c + 1) * TCH],
                                identb[0:1, 0:1])
        wcol = tiny.tile([TCH, 3, 1], BF16, tag="wcol")
        nc.vector.tensor_copy(wcol, wtp[:, :, 0:1])
        tps = pm([128, 1])
        for c in range(3):
            nc.tensor.matmul(tps, cx["hat2"][:, c, :], wcol[:, c, :],
                             start=(c == 0), stop=(c == 2))
        nc.vector.tensor_scalar(fus_t[:, s:s + 1], tps, ln2g, ln2b,
                                op0=ALU.mult, op1=ALU.add)

    def st_corr(s, cx):
        xmv = cx["xmv"]
        cvv = tiny.tile([90, 2], F32, tag="cvv")
        nc.vector.tensor_scalar(cvv, xmv[:, :, 1], float(T) / (T - 1), None,
                                op0=ALU.mult)
        cinv = tiny.tile([90, 2], F32, tag="cinv")
        csc = tiny.tile([90, 2], F32, tag="cv_nsc")
        rsqrt_newton(cinv, cvv, csc)
        nc.vector.tensor_scalar(cinv, cinv, 1e8, None, op0=ALU.min)
        wcorr = sb.tile([90, 2, 12], BF16, tag="wcorr")
        for g in range(2):
            nc.vector.tensor_scalar_mul(wcorr[:, g], cmask[:, g],
                                        cinv[:, g:g + 1])
        swps = pm([12, 512])
        for g in range(2):
            nc.tensor.matmul(swps[:, 0:T], wcorr[:, g], cx["xfm"][:, g, :],
                             start=(g == 0), stop=(g == 1))
        swsb = tiny.tile([12, T], BF16, tag="swsb")
        rsum = tiny.tile([12, 1], F32, tag="rsum")
        nc.vector.tensor_scalar(swsb, swps[:, 0:T], 1.0, 0.0, op0=ALU.mult,
                                op1=ALU.add, accum_out=rsum)
        swtp = pm([TCH, 3, 12], BF16)
        for c in range(3):
            nc.tensor.transpose(swtp[:, c, :],
                                swsb[:, c * TCH:(c + 1) * TCH],
                                identb[0:12, 0:12])
        swtm = tiny.tile([TCH, 3, 12], BF16, tag="swtm")
        nc.vector.tensor_copy(swtm, swtp)
        rsb = tiny.tile([12, 1], BF16, tag="rsb")
        nc.vector.tensor_copy(rsb, rsum)
        rsT_ps = pm([1, 12], BF16)
        nc.tensor.transpose(rsT_ps, rsb, identb[0:12, 0:12])
        rsT = tiny.tile([1, 12], BF16, tag="rsT")
        nc.vector.tensor_copy(rsT, rsT_ps)
        rsTn = tiny.tile([1, 12], BF16, tag="rsTn")
        nc.vector.tensor_scalar_mul(rsTn, rsT, -1.0 / T)
        gps = pm([12, 12])
        for c in range(3):
            nc.tensor.matmul(gps, swtm[:, c, :], swtm[:, c, :],
                             start=(c == 0), stop=False)
        nc.tensor.matmul(gps, rsTn, rsT, start=False, stop=True)
        nc.vector.tensor_copy(bm_all[:, s, :], gps)

    FRONT = [st_ld, st_proj, st_conv, st_gn, st_merge, st_qkv]

    def emit_front(p, ctxs):
        sA, sB = 2 * p, 2 * p + 1
        ctxs[sA] = {}
        ctxs[sB] = {}
        for st in FRONT:
            st(sA, ctxs[sA])
            st(sB, ctxs[sB])

    def emit_back_zip(p, nxt, ctxs):
        """Back half of pair p, optionally interleaved with front of pair
        nxt. Order keeps the Scalar engine's Exp and Gelu uses clustered
        (2 act-table loads per pair)."""
        sA, sB = 2 * p, 2 * p + 1
        st_scores(sA, ctxs[sA])
        st_scores(sB, ctxs[sB])
        if nxt is not None:
            nA, nB = 2 * nxt, 2 * nxt + 1
            ctxs[nA] = {}
            ctxs[nB] = {}
            st_ld(nA, ctxs[nA])
            st_ld(nB, ctxs[nB])
        st_av(sA, ctxs[sA])
        st_av(sB, ctxs[sB])
        st_corr(sA, ctxs[sA])
        st_corr(sB, ctxs[sB])
        if nxt is not None:
            st_proj(nA, ctxs[nA])
            st_conv(nA, ctxs[nA])
            st_proj(nB, ctxs[nB])
            st_conv(nB, ctxs[nB])
        st_out(sA, ctxs[sA])
        st_out(sB, ctxs[sB])
        if nxt is not None:
            st_gn(nA, ctxs[nA])
            st_gn(nB, ctxs[nB])
        st_ffn(sA, ctxs[sA])
        st_ffn(sB, ctxs[sB])
        if nxt is not None:
            st_merge(nA, ctxs[nA])
            st_merge(nB, ctxs[nB])
        st_pool(sA, ctxs[sA])
        st_pool(sB, ctxs[sB])
        if nxt is not None:
            st_qkv(nA, ctxs[nA])
            st_qkv(nB, ctxs[nB])
        del ctxs[sA], ctxs[sB]

    NP = S // 2
    ctxs = {}
    emit_front(0, ctxs)
    for p in range(NP):
        emit_back_zip(p, p + 1 if p + 1 < NP else None, ctxs)

    # ================= batched tail =================
    bm_dram = nc.dram_tensor("bm_scratch", [12, S, 12], F32).ap()
    dma.dma_start(out=bm_dram, in_=bm_all)
    fcv_f = W.tile([78, S], F32, tag="fcv_f")
    row_off = 0
    for i in range(12):
        n = 12 - i
        dma.dma_start(
            out=fcv_f[row_off:row_off + n, :],
            in_=bm_dram[i, :, i:12].transpose([1, 0]))
        row_off += n
    fcv = bcast("fcv", fcv_f)
    fcps = pm([64, S])
    nc.tensor.matmul(fcps, fcwk, fcv, start=True, stop=True)
    nc.scalar.activation(fus_f, fcps, AF.Gelu, bias=fcb, scale=1.0)

    fu_ps = pm([128, S])
    nc.tensor.matmul(fu_ps, fu1T[:, 0, :], fus_t, start=True, stop=False)
    nc.tensor.matmul(fu_ps, fu1T[0:64, 1, :], fus_f, start=False, stop=True)
    zfm = W.tile([128, S], BF16, tag="zfm")
    nc.vector.tensor_scalar(zfm, fu_ps, 1.0, fu1b, op0=ALU.mult, op1=ALU.add)
    ztp = pm([S, 128], BF16)
    nc.tensor.transpose(ztp, zfm, identb)
    ztm = W.tile([S, 128], F32, tag="ztm")
    nc.vector.tensor_copy(ztm, ztp)
    zst = W.tile([S, 6], F32, tag="zst")
    nc.vector.bn_stats(zst, ztm)
    zmv = W.tile([S, 2], F32, tag="zmv")
    nc.vector.bn_aggr(zmv, zst)
    zvpe = W.tile([S, 1], F32, tag="zvpe")
    nc.vector.tensor_scalar(zvpe, zmv[:, 1:2], EPS, None, op0=ALU.add)
    zrs = W.tile([S, 1], F32, tag="zrs")
    zsc = W.tile([S, 1], F32, tag="zsc")
    rsqrt_newton(zrs, zvpe, zsc)
    zhat = W.tile([S, 128], BF16, tag="zhat")
    nc.vector.tensor_scalar(zhat, ztm, zmv[:, 0:1], zrs,
                            op0=ALU.subtract, op1=ALU.mult)
    zhtp = pm([128, S], BF16)
    nc.tensor.transpose(zhtp, zhat, identb[0:S, 0:S])
    zg = W.tile([128, S], BF16, tag="zg")
    nc.scalar.activation(zg, zhtp, AF.Gelu, bias=flb, scale=flg)
    out_ps = pm([64, S])
    nc.tensor.matmul(out_ps, fu2T, zg, start=True, stop=True)
    out_sb = W.tile([64, S], BF16, tag="out_sb")
    nc.scalar.activation(out_sb, out_ps, AF.Identity, bias=fu2b, scale=1.0)
    outT_ps = pm([S, 64], BF16)
    nc.tensor.transpose(outT_ps, out_sb, identb[0:64, 0:64])
    outT = W.tile([S, 64], F32, tag="outT")
    nc.vector.tensor_copy(outT, outT_ps)
    dma.dma_start(out=out_dram, in_=outT)

    for p in reversed(pools):
        p.__exit__(None, None, None)


_PROGRAM = None


def _get_program():
    global _PROGRAM
    if _PROGRAM is None:
        _PROGRAM = build_program()
    return _PROGRAM


def kernel(**inputs):
    from concourse.bass_utils import run_bass_kernel_spmd

    nc, _ = _get_program()
    in_maps = []
    for c in range(NCORES):
        m = {}
        for name, _shape in INPUT_SPECS:
            if name == "x":
                m["x"] = np.ascontiguousarray(
                    np.asarray(inputs["x"][c * S:(c + 1) * S], dtype=np.float32))
            else:
                m[name] = np.ascontiguousarray(
                    np.asarray(inputs[name], dtype=np.float32))
        in_maps.append(m)
    res = run_bass_kernel_spmd(nc, in_maps, list(range(NCORES)))
    global LAST_RESULTS
    LAST_RESULTS = res
    out = np.concatenate([res.results[c]["out"] for c in range(NCORES)], axis=0)
    return out.astype(np.float32)


LAST_RESULTS = None
